# revision 1
# baseline (speedup 1.0000x reference)
"""GAT layer (gnn_message_passing) Trainium2 kernel.

Math (after algebraic simplification of the reference):
  v3 = W3 @ W5[:64];  v4 = W4 @ W5[64:]           # [64]
  s3 = drug_emb @ v3                               # [N_DRUG]
  s4[t] = tail_emb[t] . v4  (computed on the fly from gathered rows)
  Sb = drug_emb @ (rel_emb * (W1 @ 1)).T + sum(b1) # [N_DRUG, N_REL]
  att_e  = leaky_relu(s3[h_e] + s4[t_e])
  p_e    = exp(att_e)            (softmax max-shift dropped: shift-invariant)
  w_e    = p_e * Sb[h_e, r_e]
  U[h]   = sum_e w_e * tail_emb[t_e];  den[h] = sum_e p_e
  neigh  = U / den
  y      = [drug_emb | neigh] @ W2 + b2;  out = batchnorm(y) (training stats)

Sharding: edges sorted by head on the host (index-only preprocessing);
8 cores own disjoint 2500-head ranges, so segment stats complete locally.
Only the 64x2 batchnorm statistics are all-reduced.

Device per core: dma_gather tail rows; per-128-edge block one-hot matmuls
implement the per-edge s3/Sb gathers and the segment reduction in PSUM.
"""

import math
import os

import numpy as np

import concourse.bacc as bacc
import concourse.bass as bass
import concourse.tile as tile
from concourse import mybir
from concourse.bass_utils import run_bass_kernel_spmd

F32 = mybir.dt.float32
I16 = mybir.dt.int16
AF = mybir.ActivationFunctionType
OP = mybir.AluOpType

N_DRUG = 20000
N_TAIL = 20000
N_REL = 64
D = 64
NC = 8
HPC = N_DRUG // NC          # heads per core
WIN = 128                   # heads per window
NWIN = (HPC + WIN - 1) // WIN  # windows per core (20)
DROWS = NWIN * WIN          # padded drug rows per core (2560)
EPS = 1e-5
SLOPE = 0.01
GRP = 8                     # blocks per batched-scalar group
HROW_BLKS = 16              # blocks per hrel-row staging tile
NQ = int(os.environ.get("K_NQ", "1"))      # SWDGE queues for dma_gather
DMA_SCRATCH = int(os.environ.get("K_SCRATCH", "16384"))
GCH = int(os.environ.get("K_GCH", "8"))    # blocks per dma_gather call
USE_GATHER = os.environ.get("K_GATHER", "1") == "1"
USE_CC = os.environ.get("K_CC", "1") == "1"
PREGATHER = os.environ.get("K_PREGATHER", "0") == "1"


def _build_nc(NBW: int):
    """Build the Bass module. NBW = 128-edge blocks per 128-head window."""
    NB = NWIN * NBW          # blocks per core
    S = NB * 128             # edge slots per core

    nc = bacc.Bacc(None, num_devices=NC, num_swdge_queues=NQ,
                   dynamic_dma_scratch_size=DMA_SCRATCH)

    # ---- I/O ----
    def inp(name, shape, dtype=F32):
        return nc.declare_dram_parameter(name, list(shape), dtype, isOutput=False)

    tail_emb = inp("tail_emb", (N_TAIL, D))
    drug_rows = inp("drug_rows", (DROWS, D))
    rel_emb = inp("rel_emb", (N_REL, D))
    W1 = inp("W1", (D, D))
    W2 = inp("W2", (2 * D, D))
    W3 = inp("W3", (D, D))
    W4 = inp("W4", (D, D))
    W5 = inp("W5", (2 * D, 1))
    b1c = inp("b1c", (D, 1))
    b2r = inp("b2r", (1, D))
    gammac = inp("gammac", (D, 1))
    betac = inp("betac", (D, 1))

    hrelC = inp("hrelC", (128, NB))      # hrel (head - window base) per slot
    relC = inp("relC", (128, NB))
    maskC = inp("maskC", (128, NB))
    hrel_flat = inp("hrel_flat", (S,))   # token-major copy for row loads
    tails16 = inp("tails16", (128, S // 16), I16)  # wrapped idx, 8x replicated
    tg_rows = inp("tg_rows", (S, D)) if PREGATHER else None

    c_iota128 = inp("c_iota128", (128, 128))  # row p = 0..127
    c_iotaP = inp("c_iotaP", (128, 1))        # col = partition index
    c_iota64 = inp("c_iota64", (128, 64))
    c_id128 = inp("c_id128", (128, 128))
    c_id64 = inp("c_id64", (64, 64))
    c_ones1 = inp("c_ones1", (1, 128))
    c_ones64 = inp("c_ones64", (64, 1))
    c_ones128 = inp("c_ones128", (128, 1))

    out_rows = nc.declare_dram_parameter("out_rows", [DROWS, D], F32, isOutput=True)
    out_dbg = nc.declare_dram_parameter("out_dbg", [D, 4], F32, isOutput=True)

    # collective bounce buffers
    cc_in = nc.dram_tensor("cc_in", [D, 2], F32)
    cc_out = nc.dram_tensor("cc_out", [D, 2], F32, addr_space="Shared")

    with tile.TileContext(nc) as tc:
        cst = tc.alloc_tile_pool(name="cst", bufs=1)
        big = tc.alloc_tile_pool(name="big", bufs=1)
        sb = tc.alloc_tile_pool(name="sb", bufs=2)
        ohp = tc.alloc_tile_pool(name="ohp", bufs=GRP + 2)
        wtp = tc.alloc_tile_pool(name="wtp", bufs=2)
        tgp = tc.alloc_tile_pool(name="tgp", bufs=2)
        hrp = tc.alloc_tile_pool(name="hrp", bufs=2)
        grp = tc.alloc_tile_pool(name="grp", bufs=2)
        ps = tc.alloc_tile_pool(name="ps", bufs=2, space="PSUM")
        psG = tc.alloc_tile_pool(name="psG", bufs=2, space="PSUM")
        psU = tc.alloc_tile_pool(name="psU", bufs=2, space="PSUM")
        psS = tc.alloc_tile_pool(name="psS", bufs=1, space="PSUM")

        def body():
            # absorber: first DVE instruction after the init barrier must carry
            # no data wait (compact DVE structs have a single wait slot).
            dve0 = cst.tile([128, 1], F32, tag="dve0")
            nc.vector.memset(dve0[:], 0.0)

            def mm(out, lhsT, rhs, start=True, stop=True, is_transpose=None):
                return nc.tensor.matmul(out, lhsT, rhs, start=start, stop=stop,
                                        is_transpose=is_transpose,
                                        skip_group_check=True)

            import bass_rust as _br

            def dep(a, b):
                _br.add_dep_helper(a.ins, b.ins, sync=True, reason="wait-routing")

            def load(pool, src_ap, shape, dtype=F32, name=None):
                t = pool.tile(list(shape), dtype, tag=name)
                nc.sync.dma_start(out=t[:], in_=src_ap)
                return t

            # ---- constants into SBUF ----
            iota128 = load(cst, c_iota128[:, :], (128, 128), name="iota128")
            iotaP = load(cst, c_iotaP[:, :], (128, 1), name="iotaP")
            iota64 = load(cst, c_iota64[:, :], (128, 64), name="iota64")
            id128 = load(cst, c_id128[:, :], (128, 128), name="id128")
            id64 = load(cst, c_id64[:, :], (64, 64), name="id64")
            ones1 = load(cst, c_ones1[:, :], (1, 128), name="ones1")
            ones64 = load(cst, c_ones64[:, :], (64, 1), name="ones64")
            ones128 = load(cst, c_ones128[:, :], (128, 1), name="ones128")

            hrelC_t = load(big, hrelC[:, :], (128, NB), name="hrelC")
            relC_t = load(big, relC[:, :], (128, NB), name="relC")
            maskC_t = load(big, maskC[:, :], (128, NB), name="maskC")
            tails_t = load(big, tails16[:, :], (128, S // 16), I16, name="tails")

            w1t = load(cst, W1[:, :], (64, 64), name="w1")
            w2a = load(cst, W2[0:64, :], (64, 64), name="w2a")
            w2b = load(cst, W2[64:128, :], (64, 64), name="w2b")
            w3t = load(cst, W3[:, :], (64, 64), name="w3")
            w4t = load(cst, W4[:, :], (64, 64), name="w4")
            w5a = load(cst, W5[0:64, :], (64, 1), name="w5a")
            w5b = load(cst, W5[64:128, :], (64, 1), name="w5b")
            relt = load(cst, rel_emb[:, :], (64, 64), name="relt")
            b1col = load(cst, b1c[:, :], (64, 1), name="b1col")
            b2row = load(cst, b2r[:, :], (1, 64), name="b2row")
            gcol = load(cst, gammac[:, :], (64, 1), name="gcol")
            bcol = load(cst, betac[:, :], (64, 1), name="bcol")

            # ---- phase 1: weight folding ----
            def transpose_to(pool, src_t, k, m, name):
                # src [k, m] -> dst [m, k]  (PE transpose via identity)
                pst = ps.tile([m, k], F32, tag="ps")
                ident = id64 if k == 64 else id128
                mm(pst[:], src_t[:], ident[:, 0:k], is_transpose=True)
                dst = pool.tile([m, k], F32, tag=name)
                nc.scalar.copy(dst[:], pst[:])
                return dst

            w3T = transpose_to(cst, w3t, 64, 64, "w3T")
            w4T = transpose_to(cst, w4t, 64, 64, "w4T")
            w1T = transpose_to(cst, w1t, 64, 64, "w1T")
            relT = transpose_to(cst, relt, 64, 64, "relT")

            def mm_to_sbuf(pool, lhsT, rhs, m, n, name):
                pst = ps.tile([m, n], F32, tag="ps")
                mm(pst[:], lhsT, rhs)
                dst = pool.tile([m, n], F32, tag=name)
                nc.scalar.copy(dst[:], pst[:])
                return dst

            v3 = mm_to_sbuf(cst, w3T[:], w5a[:], 64, 1, "v3")      # [64,1]
            v4 = mm_to_sbuf(cst, w4T[:], w5b[:], 64, 1, "v4")      # [64,1]
            w1s = mm_to_sbuf(cst, w1T[:], ones64[:], 64, 1, "w1s")  # [64,1]
            b1s = mm_to_sbuf(cst, b1col[:], ones64[:], 1, 1, "b1s")  # [1,1]

            # v4 as a [128, 64] broadcast tile
            psv4r = ps.tile([1, 64], F32, tag="ps")
            mm(psv4r[:], v4[:], id64[:], is_transpose=True)
            v4row = cst.tile([1, 64], F32, tag="v4row")
            nc.scalar.copy(v4row[:], psv4r[:])
            v4tile = cst.tile([128, 64], F32, tag="v4tile")
            nc.gpsimd.partition_broadcast(v4tile[:], v4row[:])

            b1s_tile = cst.tile([128, 1], F32, tag="b1stile")
            nc.gpsimd.partition_broadcast(b1s_tile[:], b1s[:])

            b2tile = cst.tile([128, 64], F32, tag="b2tile")
            nc.gpsimd.partition_broadcast(b2tile[:], b2row[:])

            # M_T = rel_emb.T * w1s (per-partition scale)
            MT = cst.tile([64, 64], F32, tag="MT")
            nc.vector.tensor_scalar(MT[:], relT[:], w1s[:], None, OP.mult)

            # DVE fences: advance DVE's clock past the preload DMAs and the
            # gpsimd broadcasts so hot-loop TensorScalarPtr ops (1 wait slot)
            # never need more than one embedded wait.
            for fi, ft in enumerate((iota128, iotaP, iota64, hrelC_t, relC_t,
                                     maskC_t, v4tile, b2tile, gcol, bcol)):
                np_ = ft.shape[0]
                fj = cst.tile([np_, 1], F32, tag=f"fj{fi}")
                nc.vector.tensor_copy(fj[:], ft[0:np_, 0:1])
            fj16 = cst.tile([128, 1], I16, tag="fj16")
            nc.vector.tensor_copy(fj16[:], tails_t[:, 0:1])

            # ---- phase 2: per-window drug prep ----
            SW = cst.tile([128, NWIN, 65], F32, tag="SW")   # [Sb | s3] per window
            drugTs = []
            for w in range(NWIN):
                dchunk = sb.tile([128, 64], F32, tag="dchunk")
                nc.sync.dma_start(out=dchunk[:], in_=drug_rows[w * 128:(w + 1) * 128, :])
                psDT = ps.tile([64, 128], F32, tag="ps")
                mm(psDT[:], dchunk[:], id128[:], is_transpose=True)
                dT = cst.tile([64, 128], F32, tag=f"drugT{w}")
                nc.scalar.copy(dT[:], psDT[:])
                drugTs.append(dT)
                psSb = ps.tile([128, 64], F32, tag="ps")
                mm(psSb[:], dT[:], MT[:])
                nc.scalar.activation(SW[:, w, 0:64], psSb[:], AF.Identity,
                                     bias=b1s_tile[:], scale=1.0)
                psS3 = ps.tile([128, 1], F32, tag="ps")
                mm(psS3[:], dT[:], v3[:])
                nc.scalar.copy(SW[:, w, 64:65], psS3[:])

            # ---- phase 3: edge pass ----
            neigh = cst.tile([128, NWIN, 64], F32, tag="neigh")

            n_hrow = (NB + HROW_BLKS - 1) // HROW_BLKS
            for w in range(NWIN):
                # gather this window's tail rows: [128, NBW, 64]
                t_tile = tgp.tile([128, NBW, 64], F32, tag="tgath")
                nidx = NBW * 128
                if PREGATHER:
                    base = w * NBW * 128
                    src = tg_rows[base:base + NBW * 128, :].rearrange(
                        "(n p) d -> p n d", p=128)
                    gat_i = nc.sync.dma_start(out=t_tile[:], in_=src)
                elif USE_GATHER:
                    gat_i = None
                    for g0 in range(0, NBW, GCH):
                        gn = min(GCH, NBW - g0) * 128
                        io = (w * NBW + g0) * 8
                        gat_i = nc.gpsimd.dma_gather(
                            out_ap=t_tile[:, g0:g0 + gn // 128, :],
                            in_ap=tail_emb[:, :],
                            idxs_ap=tails_t[0:16, io:io + gn // 16],
                            num_idxs=gn,
                            num_idxs_reg=gn,
                            elem_size=64,
                            queue_num=(w * ((NBW + GCH - 1) // GCH)
                                       + g0 // GCH) % NQ,
                        )
                else:
                    gat_i = nc.vector.memset(t_tile[:], 0.125)

                pU = psU.tile([128, 65], F32, tag="pU")

                for j0 in range(0, NBW, GRP):
                    g = min(GRP, NBW - j0)
                    s3g = grp.tile([128, GRP], F32, tag="s3g")
                    s4g = grp.tile([128, GRP], F32, tag="s4g")
                    svg = grp.tile([128, GRP], F32, tag="svg")
                    wt8 = wtp.tile([128, GRP, 65], F32, tag="wt8")
                    ohs = []
                    for jj in range(g):
                        j = j0 + jj
                        b = w * NBW + j
                        # hrel row staging for the broadcast matmul
                        if b % HROW_BLKS == 0:
                            hrow = hrp.tile([1, HROW_BLKS * 128], F32, tag="hrow")
                            hb = min(b + HROW_BLKS, NB) * 128
                            nc.sync.dma_start(
                                out=hrow[0:1, 0:hb - b * 128],
                                in_=hrel_flat[b * 128:hb][None, :])
                        psA = ps.tile([128, 128], F32, tag="ps")
                        co = (b % HROW_BLKS) * 128
                        mm(psA[:], ones1[:], hrow[0:1, co:co + 128])
                        if b == 0:
                            shA = sb.tile([128, 1], F32, tag="shA")
                            nc.vector.tensor_copy(shA[:], psA[:, 0:1])
                        ohT = sb.tile([128, 128], F32, tag="ohT")
                        nc.vector.tensor_scalar(ohT[:], psA[:], iotaP[:], None,
                                                OP.is_equal)
                        pG = psG.tile([128, 65], F32, tag="pG")
                        pg_i = mm(pG[:], ohT[:], SW[:, w, :])
                        # ohr/oh are regular single-wait ops; route the PE (pG)
                        # and DMA (gather) waits through them so the stt ops
                        # below only need their mandatory DVE self-wait.
                        ohr = sb.tile([128, 64], F32, tag="ohr")
                        ohr_i = nc.vector.tensor_scalar(ohr[:], iota64[:],
                                                        relC_t[:, b:b + 1], None,
                                                        OP.is_equal)
                        dep(ohr_i, pg_i)
                        oh = ohp.tile([128, 128], F32, tag="oh")
                        oh_i = nc.vector.tensor_scalar(oh[:], iota128[:],
                                                       hrelC_t[:, b:b + 1], None,
                                                       OP.is_equal)
                        dep(oh_i, gat_i)
                        ohs.append(oh)
                        junk = sb.tile([128, 64], F32, tag="junk")
                        nc.vector.scalar_tensor_tensor(
                            out=junk[:], in0=ohr[:], scalar=0.0, in1=pG[:, 0:64],
                            op0=OP.bypass, op1=OP.mult,
                            accum_out=svg[:, jj:jj + 1])
                        junk2 = sb.tile([128, 64], F32, tag="junk2")
                        nc.vector.scalar_tensor_tensor(
                            out=junk2[:], in0=t_tile[:, j, :], scalar=0.0,
                            in1=v4tile[:], op0=OP.bypass, op1=OP.mult,
                            accum_out=s4g[:, jj:jj + 1])
                        nc.scalar.copy(s3g[:, jj:jj + 1], pG[:, 64:65])

                    # batched scalar pipeline for the group. Shield copies first:
                    # absorb the ACT (s3g) wait on a regular DVE op so the
                    # TensorTensor adds below carry at most one wait each.
                    shld = sb.tile([128, 1], F32, tag="shld")
                    nc.vector.tensor_copy(shld[:], s3g[:, g - 1:g])
                    attg = grp.tile([128, GRP], F32, tag="attg")
                    nc.vector.tensor_tensor(out=attg[:, 0:g], in0=s3g[:, 0:g],
                                            in1=s4g[:, 0:g], op=OP.add)
                    nc.scalar.activation(attg[:, 0:g], attg[:, 0:g], AF.Lrelu,
                                         bias=0.0, scale=1.0, alpha=SLOPE)
                    nc.scalar.activation(attg[:, 0:g], attg[:, 0:g], AF.Exp)
                    pmg = grp.tile([128, GRP], F32, tag="pmg")
                    nc.vector.tensor_tensor(out=pmg[:, 0:g], in0=attg[:, 0:g],
                                            in1=maskC_t[:, w * NBW + j0:
                                                        w * NBW + j0 + g], op=OP.mult)
                    wg = grp.tile([128, GRP], F32, tag="wg")
                    nc.vector.tensor_tensor(out=wg[:, 0:g], in0=pmg[:, 0:g],
                                            in1=svg[:, 0:g], op=OP.mult)
                    # scaled tail rows + p column
                    for jj in range(g):
                        j = j0 + jj
                        nc.vector.tensor_scalar(wt8[:, jj, 0:64], t_tile[:, j, :],
                                                wg[:, jj:jj + 1], None, OP.mult)
                    nc.vector.tensor_copy(wt8[:, 0:g, 64], pmg[:, 0:g])
                    for jj in range(g):
                        j = j0 + jj
                        mm(pU[:], ohs[jj][:], wt8[:, jj, :],
                           start=(j == 0), stop=(j == NBW - 1))

                # window reduction -> neigh
                dsafe = sb.tile([128, 1], F32, tag="dsafe")
                nc.vector.tensor_scalar(dsafe[:], pU[:, 64:65], 1e-30, None, OP.add)
                recip = sb.tile([128, 1], F32, tag="recip")
                nc.vector.reciprocal(recip[:], dsafe[:])
                nc.vector.tensor_scalar(neigh[:, w, :], pU[:, 0:64], recip[:], None,
                                        OP.mult)

            # ---- phase 4: output head + batchnorm ----
            ybuf = cst.tile([128, NWIN, 64], F32, tag="ybuf")
            pStat0 = psS.tile([64, 1], F32, tag="pStat0")
            pStat1 = psS.tile([64, 1], F32, tag="pStat1")
            for w in range(NWIN):
                psNT = ps.tile([64, 128], F32, tag="ps")
                mm(psNT[:], neigh[:, w, :], id128[:], is_transpose=True)
                nT = sb.tile([64, 128], F32, tag="nT")
                nc.scalar.copy(nT[:], psNT[:])
                pY = ps.tile([128, 64], F32, tag="ps")
                mm(pY[:], drugTs[w][:], w2a[:], start=True, stop=False)
                mm(pY[:], nT[:], w2b[:], start=False, stop=True)
                nc.vector.tensor_tensor(out=ybuf[:, w, :], in0=pY[:], in1=b2tile[:],
                                        op=OP.add)
                sq = sb.tile([128, 64], F32, tag="sq")
                nc.scalar.square(sq[:], ybuf[:, w, :])
                mm(pStat0[:], ybuf[:, w, :], ones128[:],
                   start=(w == 0), stop=(w == NWIN - 1))
                mm(pStat1[:], sq[:], ones128[:],
                   start=(w == 0), stop=(w == NWIN - 1))

            statsb = sb.tile([64, 2], F32, tag="statsb")
            nc.scalar.copy(statsb[:, 0:1], pStat0[:])
            nc.scalar.copy(statsb[:, 1:2], pStat1[:])
            nc.sync.dma_start(out=cc_in[:, :], in_=statsb[:])
            if USE_CC:
                nc.gpsimd.collective_compute(
                    "AllReduce", OP.add, replica_groups=[list(range(NC))],
                    ins=[cc_in[:, :]], outs=[cc_out[:, :]])
            else:
                nc.sync.dma_start(out=cc_out[:, :], in_=cc_in[:, :])
            statsg = sb.tile([64, 2], F32, tag="statsg")
            nc.sync.dma_start(out=statsg[:], in_=cc_out[:, :])
            fjs = sb.tile([64, 1], F32, tag="fjs")
            nc.vector.tensor_copy(fjs[:], statsg[:, 0:1])
            nc.sync.dma_start(out=out_dbg[:, 0:2], in_=statsb[:])
            nc.sync.dma_start(out=out_dbg[:, 2:4], in_=statsg[:])

            mean = sb.tile([64, 1], F32, tag="mean")
            nc.vector.tensor_scalar(mean[:], statsg[:, 0:1], 1.0 / N_DRUG, None, OP.mult)
            ex2 = sb.tile([64, 1], F32, tag="ex2")
            nc.vector.tensor_scalar(ex2[:], statsg[:, 1:2], 1.0 / N_DRUG, None, OP.mult)
            msq = sb.tile([64, 1], F32, tag="msq")
            nc.vector.tensor_tensor(out=msq[:], in0=mean[:], in1=mean[:], op=OP.mult)
            var = sb.tile([64, 1], F32, tag="var")
            nc.vector.tensor_tensor(out=var[:], in0=ex2[:], in1=msq[:], op=OP.subtract)
            vare = sb.tile([64, 1], F32, tag="vare")
            nc.vector.tensor_scalar(vare[:], var[:], EPS, None, OP.add)
            sd = sb.tile([64, 1], F32, tag="sd")
            nc.scalar.activation(sd[:], vare[:], AF.Sqrt)
            rstd = sb.tile([64, 1], F32, tag="rstd")
            nc.vector.reciprocal(rstd[:], sd[:])
            scalec = sb.tile([64, 1], F32, tag="scalec")
            nc.vector.tensor_tensor(out=scalec[:], in0=gcol[:], in1=rstd[:], op=OP.mult)
            tmp = sb.tile([64, 1], F32, tag="tmp")
            nc.vector.tensor_tensor(out=tmp[:], in0=mean[:], in1=scalec[:], op=OP.mult)
            shiftc = sb.tile([64, 1], F32, tag="shiftc")
            nc.vector.tensor_tensor(out=shiftc[:], in0=bcol[:], in1=tmp[:],
                                    op=OP.subtract)

            def col_to_tile(col, name):
                pst = ps.tile([1, 64], F32, tag="ps")
                mm(pst[:], col[:], id64[:], is_transpose=True)
                row = sb.tile([1, 64], F32, tag=name + "r")
                nc.scalar.copy(row[:], pst[:])
                t = cst.tile([128, 64], F32, tag=name)
                nc.gpsimd.partition_broadcast(t[:], row[:])
                return t

            scale_t = col_to_tile(scalec, "scalet")
            shift_t = col_to_tile(shiftc, "shiftt")
            for fi, ft in enumerate((scale_t, shift_t)):
                fjt = sb.tile([128, 1], F32, tag=f"fjt{fi}")
                nc.vector.tensor_copy(fjt[:], ft[:, 0:1])

            for w in range(NWIN):
                o1 = sb.tile([128, 64], F32, tag="o1")
                nc.vector.tensor_tensor(out=o1[:], in0=ybuf[:, w, :], in1=scale_t[:],
                                        op=OP.mult)
                o2 = sb.tile([128, 64], F32, tag="o2")
                nc.vector.tensor_tensor(out=o2[:], in0=o1[:], in1=shift_t[:], op=OP.add)
                nc.sync.dma_start(out=out_rows[w * 128:(w + 1) * 128, :], in_=o2[:])


        for _rep in range(int(os.environ.get('BASS_REPEAT', '1'))):
            body()

        for p in (psS, psU, psG, ps, grp, hrp, tgp, wtp, ohp, sb, big, cst):
            p.release()

    nc.finalize()
    return nc


def _host_prep(DKG):
    """Sort edges by head, shard by head range, build per-core slot arrays."""
    heads = np.asarray(DKG[:, 0], dtype=np.int64)
    tails = np.asarray(DKG[:, 1], dtype=np.int64)
    rels = np.asarray(DKG[:, 2], dtype=np.int64)

    order = np.argsort(heads, kind="stable")
    hs, ts, rs = heads[order], tails[order], rels[order]

    core_lo = np.searchsorted(hs, HPC * np.arange(NC), side="left")
    core_hi = np.searchsorted(hs, HPC * (np.arange(NC) + 1), side="left")

    # window edge counts -> NBW
    winb = np.searchsorted(hs, WIN * np.arange(NC * NWIN), side="left")
    wine = np.searchsorted(hs, WIN * (np.arange(NC * NWIN) + 1), side="left")
    maxw = int((wine - winb).max())
    NBW = max(1, (maxw + 127) // 128)
    NB = NWIN * NBW
    S = NB * 128

    per_core = []
    for c in range(NC):
        lo, hi = core_lo[c], core_hi[c]
        ch, ct, cr = hs[lo:hi], ts[lo:hi], rs[lo:hi]
        hrel = np.zeros(S, np.float32)
        rel = np.zeros(S, np.float32)
        mask = np.zeros(S, np.float32)
        tail = np.zeros(S, np.int64)
        base = c * HPC
        for w in range(NWIN):
            wl = np.searchsorted(ch, base + w * WIN, side="left")
            wh = np.searchsorted(ch, base + (w + 1) * WIN, side="left")
            n = wh - wl
            o = w * NBW * 128
            hrel[o:o + n] = (ch[wl:wh] - base - w * WIN).astype(np.float32)
            rel[o:o + n] = cr[wl:wh].astype(np.float32)
            mask[o:o + n] = 1.0
            tail[o:o + n] = ct[wl:wh]
        hrelC = hrel.reshape(NB, 128).T.copy()
        relC = rel.reshape(NB, 128).T.copy()
        maskC = mask.reshape(NB, 128).T.copy()
        t16 = tail.reshape(S // 16, 16).T.astype(np.int16)          # [16, S/16]
        t16r = np.tile(t16, (8, 1)).copy()                          # [128, S/16]
        per_core.append(dict(hrelC=hrelC, relC=relC, maskC=maskC,
                             hrel_flat=hrel, tails16=t16r, tails_flat=tail))
    return NBW, per_core


def prepare(X, DKG, drug_emb, rel_emb, tail_emb, W1, b1, W2, b2, gamma, beta,
            W3, W4, W5):
    f = np.float32
    NBW, per_core = _host_prep(np.asarray(DKG))
    nc = _build_nc(NBW)

    consts = dict(
        c_iota128=np.broadcast_to(np.arange(128, dtype=f), (128, 128)).copy(),
        c_iotaP=np.arange(128, dtype=f).reshape(128, 1).copy(),
        c_iota64=np.broadcast_to(np.arange(64, dtype=f), (128, 64)).copy(),
        c_id128=np.eye(128, dtype=f),
        c_id64=np.eye(64, dtype=f),
        c_ones1=np.ones((1, 128), f),
        c_ones64=np.ones((64, 1), f),
        c_ones128=np.ones((128, 1), f),
    )
    weights = dict(
        tail_emb=np.asarray(tail_emb, f),
        rel_emb=np.asarray(rel_emb, f),
        W1=np.asarray(W1, f), W2=np.asarray(W2, f), W3=np.asarray(W3, f),
        W4=np.asarray(W4, f), W5=np.asarray(W5, f),
        b1c=np.asarray(b1, f).reshape(D, 1),
        b2r=np.asarray(b2, f).reshape(1, D),
        gammac=np.asarray(gamma, f).reshape(D, 1),
        betac=np.asarray(beta, f).reshape(D, 1),
    )
    de = np.asarray(drug_emb, f)
    in_maps = []
    for c in range(NC):
        dr = np.zeros((DROWS, D), f)
        dr[:HPC] = de[c * HPC:(c + 1) * HPC]
        m = dict(weights)
        m.update(consts)
        m["drug_rows"] = dr
        pc = per_core[c]
        m["hrelC"] = pc["hrelC"]
        m["relC"] = pc["relC"]
        m["maskC"] = pc["maskC"]
        m["hrel_flat"] = pc["hrel_flat"]
        m["tails16"] = pc["tails16"]
        if PREGATHER:
            m["tg_rows"] = np.ascontiguousarray(
                np.asarray(tail_emb, f)[pc["tails_flat"]])
        in_maps.append(m)
    return nc, in_maps


def kernel(X, DKG, drug_emb, rel_emb, tail_emb, W1, b1, W2, b2, gamma, beta,
           W3, W4, W5):
    X = np.asarray(X)
    nc, in_maps = prepare(X, DKG, drug_emb, rel_emb, tail_emb, W1, b1, W2, b2,
                          gamma, beta, W3, W4, W5)

    res = run_bass_kernel_spmd(nc, in_maps, core_ids=list(range(NC)))
    global LAST_RESULT
    LAST_RESULT = res
    out = np.concatenate([res.results[c]["out_rows"][:HPC] for c in range(NC)],
                         axis=0)
    return out, X


LAST_RESULT = None



# revision 13
# speedup vs baseline: 1.0997x; 1.0997x over previous
"""GAT layer (gnn_message_passing) Trainium2 kernel — v2.

Math (after algebraic simplification of the reference):
  v3 = W3 @ W5[:64];  v4 = W4 @ W5[64:]           # [64]
  s3 = drug_emb @ v3                               # [N_DRUG]
  s4 = tail_emb @ v4                               # [N_TAIL]
  Sb = drug_emb @ (rel_emb * (W1 @ 1)).T + sum(b1) # [N_DRUG, N_REL]
  att_e  = leaky_relu(s3[h_e] + s4[t_e])
  p_e    = exp(att_e)            (softmax max-shift dropped: shift-invariant)
  w_e    = p_e * Sb[h_e, r_e]
  U[h]   = sum_e w_e * tail_emb[t_e];  den[h] = sum_e p_e
  neigh  = U / den
  y      = [drug_emb | neigh] @ W2 + b2;  out = batchnorm(y) (training stats)

Sharding: edges sorted by head on the host (index-only preprocessing);
8 cores own disjoint 2500-head ranges, so segment stats complete locally.
Only the 64x2 batchnorm statistics are all-reduced.

v2 device strategy (per core, per 128-head window, 128-edge blocks):
  - tailB DRAM table [20480, 128] bf16 rows [tail|s4|pad], built on device
    once; per-window dma_gather pulls 256B rows (s4 rides along).
  - host ships one-hot matrices as fp8: ohT (head one-hot, lhsT for the
    per-edge SW-row gather matmul) and relOH (rel one-hot for Sb column
    selection).
  - per block: 1 pG matmul (gather [Sb_row|s3] per edge), 1 fused
    tensor_scalar builds the wg-scaled scatter one-hot, 1 pU matmul
    accumulates [U|den] in PSUM.  den uses rhs col64 = 1/sv so that
    wg*(1/sv) = p.
  - per 4 blocks: one TT-mult + tensor_reduce extracts sv; one TT-add
    forms att.  Per 16 blocks: lrelu (on DVE), exp (ACT), reciprocal etc.
  - all edge-pass matmuls bf16/fp8 (single HW pass vs fp32's two).
"""

import os

import numpy as np

import concourse.bacc as bacc
import concourse.bass as bass
import concourse.tile as tile
from concourse import mybir
from concourse.bass_utils import run_bass_kernel_spmd

F32 = mybir.dt.float32
BF16 = mybir.dt.bfloat16
F8 = mybir.dt.float8e4
I16 = mybir.dt.int16
AF = mybir.ActivationFunctionType
OP = mybir.AluOpType

N_DRUG = 20000
N_TAIL = 20000
N_REL = 64
D = 64
NC = 8
HPC = N_DRUG // NC          # heads per core
WIN = 128                   # heads per window
NWIN = (HPC + WIN - 1) // WIN  # windows per core (20)
DROWS = NWIN * WIN          # padded drug rows per core (2560)
TROWS = 20480               # padded tail rows (160 chunks of 128)
TCH = 16                    # tail-table chunks per iteration
EPS = 1e-5
SLOPE = 0.01
GEXT = 4                    # blocks per extraction chunk (PSUM-bank bound)
GRP = 16                    # blocks per batched-scalar group
NQ = int(os.environ.get("K_NQ", "1"))      # SWDGE queues for dma_gather
GCH = int(os.environ.get("K_GCH", "8"))    # blocks per dma_gather call
SIMIDX = os.environ.get("K_SIMIDX", "0") == "1"  # full-128 idx AP (CoreSim)
DMA_SCRATCH = int(os.environ.get("K_SCRATCH", "16384"))
PAD_H = 999.0               # hrel sentinel for padded slots (no one-hot match)
F8OHT = os.environ.get("K_F8OHT", "0") == "1"   # ship ohT as fp8 (else bf16)
F8REL = os.environ.get("K_F8REL", "0") == "1"   # ship relOH as fp8 (else bf16)
OHT_DT = F8 if F8OHT else BF16
REL_DT = F8 if F8REL else BF16


def _build_nc(NBW: int):
    """Build the Bass module. NBW = 128-edge blocks per 128-head window."""
    NB = NWIN * NBW          # blocks per core
    S = NB * 128             # edge slots per core

    nc = bacc.Bacc(None, num_devices=NC, num_swdge_queues=NQ,
                   dynamic_dma_scratch_size=DMA_SCRATCH)

    # ---- I/O ----
    def inp(name, shape, dtype=F32):
        return nc.declare_dram_parameter(name, list(shape), dtype, isOutput=False)

    tail_pad = inp("tail_pad", (TROWS, D))
    drug_rows = inp("drug_rows", (DROWS, D))
    rel_emb = inp("rel_emb", (N_REL, D))
    W1 = inp("W1", (D, D))
    W2 = inp("W2", (2 * D, D))
    W3 = inp("W3", (D, D))
    W4 = inp("W4", (D, D))
    W5 = inp("W5", (2 * D, 1))
    b1c = inp("b1c", (D, 1))
    b2r = inp("b2r", (1, D))
    gammac = inp("gammac", (D, 1))
    betac = inp("betac", (D, 1))

    hrelC = inp("hrelC", (128, NB))          # hrel per slot (pads = PAD_H)
    tails16 = inp("tails16", (128, S // 16), I16)
    ohT_in = inp("ohT", (128, NB, 128), OHT_DT)  # head one-hot [head, blk, edge]
    relOH_in = inp("relOH", (128, NB, 64), REL_DT)  # rel one-hot [edge, blk, rel]

    c_iota128b = inp("c_iota128b", (128, 128), BF16)
    c_id128 = inp("c_id128", (128, 128))
    c_id64 = inp("c_id64", (64, 64))
    c_id128b = inp("c_id128b", (128, 128), BF16)
    c_ones64 = inp("c_ones64", (64, 1))
    c_ones128b = inp("c_ones128b", (128, 1), BF16)

    out_rows = nc.declare_dram_parameter("out_rows", [DROWS, D], F32, isOutput=True)

    # DRAM scratch: bf16 tail table rows [tail(64) | s4 | junk(63)]
    tailB = nc.dram_tensor("tailB", [TROWS, 128], BF16)

    # collective bounce buffers
    cc_in = nc.dram_tensor("cc_in", [D, 2], F32)
    cc_out = nc.dram_tensor("cc_out", [D, 2], F32, addr_space="Shared")

    with tile.TileContext(nc) as tc:
        cst = tc.alloc_tile_pool(name="cst", bufs=1)
        sb = tc.alloc_tile_pool(name="sb", bufs=2)
        tbp = tc.alloc_tile_pool(name="tbp", bufs=2)
        tgp = tc.alloc_tile_pool(name="tgp", bufs=2)
        ohtp = tc.alloc_tile_pool(name="ohtp", bufs=2)
        relp = tc.alloc_tile_pool(name="relp", bufs=2)
        swp = tc.alloc_tile_pool(name="swp", bufs=2)
        grp = tc.alloc_tile_pool(name="grp", bufs=2)
        selp = tc.alloc_tile_pool(name="selp", bufs=2)
        ohsp = tc.alloc_tile_pool(name="ohsp", bufs=4)
        ps = tc.alloc_tile_pool(name="ps", bufs=1, space="PSUM")
        psG = tc.alloc_tile_pool(name="psG", bufs=2, space="PSUM")
        psU = tc.alloc_tile_pool(name="psU", bufs=2, space="PSUM")
        psS = tc.alloc_tile_pool(name="psS", bufs=1, space="PSUM")

        def body():
            # absorber: first DVE instruction after the init barrier must
            # carry no data wait.
            dve0 = cst.tile([128, 1], F32, tag="dve0")
            nc.vector.memset(dve0[:], 0.0)

            def mm(out, lhsT, rhs, start=True, stop=True, is_transpose=None):
                return nc.tensor.matmul(out, lhsT, rhs, start=start, stop=stop,
                                        is_transpose=is_transpose,
                                        skip_group_check=True)

            import bass_rust as _br

            def dep(a, b):
                _br.add_dep_helper(a.ins, b.ins, sync=True, reason="wait-routing")

            def load(pool, src_ap, shape, dtype=F32, name=None):
                t = pool.tile(list(shape), dtype, tag=name)
                nc.sync.dma_start(out=t[:], in_=src_ap)
                return t

            # ---- constants into SBUF ----
            iota128b = load(cst, c_iota128b[:, :], (128, 128), BF16, "iota128b")
            id128 = load(cst, c_id128[:, :], (128, 128), name="id128")
            id64 = load(cst, c_id64[:, :], (64, 64), name="id64")
            id128b = load(cst, c_id128b[:, :], (128, 128), BF16, "id128b")
            ones64 = load(cst, c_ones64[:, :], (64, 1), name="ones64")
            ones128b = load(cst, c_ones128b[:, :], (128, 1), BF16, "ones128b")

            hrelC_t = load(cst, hrelC[:, :], (128, NB), name="hrelC")
            tails_t = load(cst, tails16[:, :], (128, S // 16), I16, name="tails")

            w1t = load(cst, W1[:, :], (64, 64), name="w1")
            w2a = load(cst, W2[0:64, :], (64, 64), name="w2a")
            w2b = load(cst, W2[64:128, :], (64, 64), name="w2b")
            w3t = load(cst, W3[:, :], (64, 64), name="w3")
            w4t = load(cst, W4[:, :], (64, 64), name="w4")
            w5a = load(cst, W5[0:64, :], (64, 1), name="w5a")
            w5b = load(cst, W5[64:128, :], (64, 1), name="w5b")
            relt = load(cst, rel_emb[:, :], (64, 64), name="relt")
            b1col = load(cst, b1c[:, :], (64, 1), name="b1col")
            b2row = load(cst, b2r[:, :], (1, 64), name="b2row")
            gcol = load(cst, gammac[:, :], (64, 1), name="gcol")
            bcol = load(cst, betac[:, :], (64, 1), name="bcol")

            # ---- phase 1: weight folding (fp32, tiny) ----
            def transpose_to(pool, src_t, k, m, name):
                pst = ps.tile([m, k], F32, tag="ps")
                ident = id64 if k == 64 else id128
                mm(pst[:], src_t[:], ident[:, 0:k], is_transpose=True)
                dst = pool.tile([m, k], F32, tag=name)
                nc.scalar.copy(dst[:], pst[:])
                return dst

            w3T = transpose_to(cst, w3t, 64, 64, "w3T")
            w4T = transpose_to(cst, w4t, 64, 64, "w4T")
            w1T = transpose_to(cst, w1t, 64, 64, "w1T")
            relT = transpose_to(cst, relt, 64, 64, "relT")

            def mm_to_sbuf(pool, lhsT, rhs, m, n, name, dtype=F32):
                pst = ps.tile([m, n], F32, tag="ps")
                mm(pst[:], lhsT, rhs)
                dst = pool.tile([m, n], dtype, tag=name)
                nc.scalar.copy(dst[:], pst[:])
                return dst

            v3b = mm_to_sbuf(cst, w3T[:], w5a[:], 64, 1, "v3b", BF16)   # [64,1]
            v4 = mm_to_sbuf(cst, w4T[:], w5b[:], 64, 1, "v4")           # [64,1]
            w1s = mm_to_sbuf(cst, w1T[:], ones64[:], 64, 1, "w1s")      # [64,1]
            b1s = mm_to_sbuf(cst, b1col[:], ones64[:], 1, 1, "b1s")     # [1,1]

            # v4 broadcast tile [128, 64] fp32 (for the tail-table build)
            psv4r = ps.tile([1, 64], F32, tag="ps")
            mm(psv4r[:], v4[:], id64[:], is_transpose=True)
            v4row = cst.tile([1, 64], F32, tag="v4row")
            nc.scalar.copy(v4row[:], psv4r[:])
            v4tile = cst.tile([128, 64], F32, tag="v4tile")
            nc.gpsimd.partition_broadcast(v4tile[:], v4row[:])

            b1s_tile = cst.tile([128, 1], F32, tag="b1stile")
            nc.gpsimd.partition_broadcast(b1s_tile[:], b1s[:])

            b2tile = cst.tile([128, 64], F32, tag="b2tile")
            nc.gpsimd.partition_broadcast(b2tile[:], b2row[:])

            # bf16 weights for the edge/output matmuls
            w2ab = cst.tile([64, 64], BF16, tag="w2ab")
            nc.scalar.copy(w2ab[:], w2a[:])
            w2bb = cst.tile([64, 64], BF16, tag="w2bb")
            nc.scalar.copy(w2bb[:], w2b[:])

            # M_T = rel_emb.T * w1s (per-partition scale), bf16
            MTb = cst.tile([64, 64], BF16, tag="MTb")
            nc.vector.tensor_scalar(MTb[:], relT[:], w1s[:], None, OP.mult)

            # DVE fences: absorb preload DMA waits so hot-loop compact DVE
            # ops never need more than one embedded wait.
            for fi, ft in enumerate((iota128b, hrelC_t, v4tile, b2tile,
                                     gcol, bcol)):
                np_ = ft.shape[0]
                fj = cst.tile([np_, 1], F32, tag=f"fj{fi}")
                nc.vector.tensor_copy(fj[:], ft[0:np_, 0:1])
            fj16 = cst.tile([128, 1], I16, tag="fj16")
            nc.vector.tensor_copy(fj16[:], tails_t[:, 0:1])

            # ---- phase 1b: tail table build ----
            NIT = TROWS // (TCH * 128)
            for it in range(NIT):
                base = it * TCH * 128
                tch = tbp.tile([128, TCH, 64], F32, tag="tch")
                src = tail_pad[base:base + TCH * 128, :].rearrange(
                    "(c p) d -> p c d", p=128)
                nc.sync.dma_start(out=tch[:], in_=src)
                tbb = tbp.tile([128, TCH, 128], BF16, tag="tbb")
                nc.vector.memset(tbb[:, :, 65:128], 0.0)
                s4f = tbp.tile([128, TCH], F32, tag="s4f")
                for c in range(TCH):
                    junk = sb.tile([128, 64], F32, tag="junk")
                    nc.vector.scalar_tensor_tensor(
                        out=junk[:], in0=tch[:, c, :], scalar=0.0,
                        in1=v4tile[:], op0=OP.bypass, op1=OP.mult,
                        accum_out=s4f[:, c:c + 1])
                nc.vector.tensor_copy(tbb[:, :, 0:64], tch[:])
                nc.vector.tensor_copy(tbb[:, :, 64], s4f[:])
                dst = tailB[base:base + TCH * 128, :].rearrange(
                    "(c p) d -> p c d", p=128)
                nc.sync.dma_start(out=dst, in_=tbb[:])

            # ---- phase 2+3 fused: per-window pipeline ----
            ybuf = cst.tile([128, NWIN, 64], BF16, tag="ybuf")
            pStat = psS.tile([64, 2], F32, tag="pStat")
            drugTs = []

            NCH = (NBW + GCH - 1) // GCH
            for w in range(NWIN):
                # window drug prep -> SW = [Sb | s3] bf16 [128, 65]
                dchunk = sb.tile([128, 64], F32, tag="dchunk")
                nc.sync.dma_start(out=dchunk[:],
                                  in_=drug_rows[w * 128:(w + 1) * 128, :])
                psDT = ps.tile([64, 128], F32, tag="ps")
                mm(psDT[:], dchunk[:], id128[:], is_transpose=True)
                dTb = cst.tile([64, 128], BF16, tag=f"drugT{w}")
                nc.scalar.copy(dTb[:], psDT[:])
                drugTs.append(dTb)
                SW = swp.tile([128, 65], BF16, tag="SW")
                psSb = ps.tile([128, 64], F32, tag="ps")
                mm(psSb[:], dTb[:], MTb[:])
                nc.scalar.activation(SW[:, 0:64], psSb[:], AF.Identity,
                                     bias=b1s_tile[:], scale=1.0)
                psS3 = ps.tile([128, 1], F32, tag="ps")
                mm(psS3[:], dTb[:], v3b[:])
                nc.scalar.copy(SW[:, 64:65], psS3[:])

                # gather this window's tail rows [128, NBW, 128] bf16
                tg = tgp.tile([128, NBW, 128], BF16, tag="tg")
                gat_i = None
                for g0 in range(0, NBW, GCH):
                    gn = min(GCH, NBW - g0) * 128
                    io = (w * NBW + g0) * 8
                    gat_i = nc.gpsimd.dma_gather(
                        out_ap=tg[:, g0:g0 + gn // 128, :],
                        in_ap=tailB[:, :],
                        idxs_ap=(tails_t[:, io:io + gn // 16] if SIMIDX
                                 else tails_t[0:16, io:io + gn // 16]),
                        num_idxs=gn,
                        num_idxs_reg=gn,
                        elem_size=128,
                        queue_num=(w * NCH + g0 // GCH) % NQ,
                    )

                # stream this window's one-hots
                ohTw = ohtp.tile([128, NBW, 128], OHT_DT, tag="ohTw")
                nc.sync.dma_start(out=ohTw[:],
                                  in_=ohT_in[:, w * NBW:(w + 1) * NBW, :])
                relw = relp.tile([128, NBW, 64], REL_DT, tag="relw")
                nc.sync.dma_start(out=relw[:],
                                  in_=relOH_in[:, w * NBW:(w + 1) * NBW, :])

                pU = psU.tile([128, 65], F32, tag="pU")

                for j0 in range(0, NBW, GRP):
                    g = min(GRP, NBW - j0)
                    sv16 = grp.tile([128, GRP], F32, tag="sv16")
                    att16 = grp.tile([128, GRP], F32, tag="att16")
                    s4c16 = grp.tile([128, GRP], F32, tag="s4c16")
                    # s4 junction copy (absorbs the gather DMA wait)
                    s4i = nc.scalar.copy(s4c16[:, 0:g], tg[:, j0:j0 + g, 64])
                    dep(s4i, gat_i)
                    for q in range(0, g, GEXT):
                        psG4 = psG.tile([128, GEXT, 65], F32, tag="psG4")
                        for i in range(GEXT):
                            j = j0 + q + i
                            mm(psG4[:, i, :], ohTw[:, j, :], SW[:],
                               start=True, stop=True)
                        sel4 = selp.tile([128, GEXT, 64], BF16, tag="sel4")
                        nc.vector.tensor_tensor(
                            out=sel4[:], in0=psG4[:, :, 0:64],
                            in1=relw[:, j0 + q:j0 + q + GEXT, :], op=OP.mult)
                        nc.vector.tensor_reduce(
                            out=sv16[:, q:q + GEXT], in_=sel4[:],
                            axis=mybir.AxisListType.X, op=OP.add)
                        nc.vector.tensor_tensor(
                            out=att16[:, q:q + GEXT], in0=psG4[:, :, 64],
                            in1=s4c16[:, q:q + GEXT], op=OP.add)
                    # group scalar pipeline [128, g]
                    l16 = grp.tile([128, GRP], F32, tag="l16")
                    nc.vector.scalar_tensor_tensor(
                        out=l16[:, 0:g], in0=att16[:, 0:g], scalar=SLOPE,
                        in1=att16[:, 0:g], op0=OP.mult, op1=OP.max)
                    p16 = grp.tile([128, GRP], F32, tag="p16")
                    nc.scalar.activation(p16[:, 0:g], l16[:, 0:g], AF.Exp)
                    svc16 = grp.tile([128, GRP], F32, tag="svc16")
                    nc.vector.tensor_scalar(svc16[:, 0:g], sv16[:, 0:g], 1e-12,
                                            None, OP.add)
                    wg16 = grp.tile([128, GRP], F32, tag="wg16")
                    nc.vector.tensor_tensor(out=wg16[:, 0:g], in0=p16[:, 0:g],
                                            in1=svc16[:, 0:g], op=OP.mult)
                    rc16 = grp.tile([128, GRP], F32, tag="rc16")
                    nc.vector.reciprocal(rc16[:, 0:g], svc16[:, 0:g])
                    nc.scalar.copy(tg[:, j0:j0 + g, 64], rc16[:, 0:g])
                    for jj in range(g):
                        j = j0 + jj
                        ohS = ohsp.tile([128, 128], BF16, tag="ohS")
                        nc.vector.tensor_scalar(
                            ohS[:], iota128b[:],
                            hrelC_t[:, w * NBW + j:w * NBW + j + 1],
                            wg16[:, jj:jj + 1], OP.is_equal, OP.mult)
                        mm(pU[:], ohS[:], tg[:, j, 0:65],
                           start=(j == 0), stop=(j == NBW - 1))

                # window reduction -> neigh -> y -> stats
                dsafe = sb.tile([128, 1], F32, tag="dsafe")
                nc.vector.tensor_scalar(dsafe[:], pU[:, 64:65], 1e-30, None,
                                        OP.add)
                recip = sb.tile([128, 1], F32, tag="recip")
                nc.vector.reciprocal(recip[:], dsafe[:])
                nw = sb.tile([128, 64], F32, tag="nw")
                nc.vector.tensor_scalar(nw[:], pU[:, 0:64], recip[:], None,
                                        OP.mult)
                psNT = ps.tile([64, 128], F32, tag="psT")
                mm(psNT[:], nw[:], id128[:], is_transpose=True)
                nT = sb.tile([64, 128], BF16, tag="nT")
                nc.scalar.copy(nT[:], psNT[:])
                pY = ps.tile([128, 64], F32, tag="ps")
                mm(pY[:], drugTs[w][:], w2ab[:], start=True, stop=False)
                mm(pY[:], nT[:], w2bb[:], start=False, stop=True)
                nc.vector.tensor_tensor(out=ybuf[:, w, :], in0=pY[:],
                                        in1=b2tile[:], op=OP.add)
                sq = sb.tile([128, 64], BF16, tag="sq")
                nc.vector.tensor_tensor(out=sq[:], in0=ybuf[:, w, :],
                                        in1=ybuf[:, w, :], op=OP.mult)
                mm(pStat[:, 0:1], ybuf[:, w, :], ones128b[:],
                   start=(w == 0), stop=(w == NWIN - 1))
                mm(pStat[:, 1:2], sq[:], ones128b[:],
                   start=(w == 0), stop=(w == NWIN - 1))

            # ---- phase 4: batchnorm ----
            statsb = sb.tile([64, 2], F32, tag="statsb")
            nc.scalar.copy(statsb[:], pStat[:])
            nc.sync.dma_start(out=cc_in[:, :], in_=statsb[:])
            nc.gpsimd.collective_compute(
                "AllReduce", OP.add, replica_groups=[list(range(NC))],
                ins=[cc_in[:, :]], outs=[cc_out[:, :]])
            statsg = sb.tile([64, 2], F32, tag="statsg")
            nc.sync.dma_start(out=statsg[:], in_=cc_out[:, :])
            fjs = sb.tile([64, 1], F32, tag="fjs")
            nc.vector.tensor_copy(fjs[:], statsg[:, 0:1])

            mean = sb.tile([64, 1], F32, tag="mean")
            nc.vector.tensor_scalar(mean[:], statsg[:, 0:1], 1.0 / N_DRUG,
                                    None, OP.mult)
            ex2 = sb.tile([64, 1], F32, tag="ex2")
            nc.vector.tensor_scalar(ex2[:], statsg[:, 1:2], 1.0 / N_DRUG,
                                    None, OP.mult)
            msq = sb.tile([64, 1], F32, tag="msq")
            nc.vector.tensor_tensor(out=msq[:], in0=mean[:], in1=mean[:],
                                    op=OP.mult)
            var = sb.tile([64, 1], F32, tag="var")
            nc.vector.tensor_tensor(out=var[:], in0=ex2[:], in1=msq[:],
                                    op=OP.subtract)
            vare = sb.tile([64, 1], F32, tag="vare")
            nc.vector.tensor_scalar(vare[:], var[:], EPS, None, OP.add)
            sd = sb.tile([64, 1], F32, tag="sd")
            nc.scalar.activation(sd[:], vare[:], AF.Sqrt)
            rstd = sb.tile([64, 1], F32, tag="rstd")
            nc.vector.reciprocal(rstd[:], sd[:])
            scalec = sb.tile([64, 1], F32, tag="scalec")
            nc.vector.tensor_tensor(out=scalec[:], in0=gcol[:], in1=rstd[:],
                                    op=OP.mult)
            tmp = sb.tile([64, 1], F32, tag="tmp")
            nc.vector.tensor_tensor(out=tmp[:], in0=mean[:], in1=scalec[:],
                                    op=OP.mult)
            shiftc = sb.tile([64, 1], F32, tag="shiftc")
            nc.vector.tensor_tensor(out=shiftc[:], in0=bcol[:], in1=tmp[:],
                                    op=OP.subtract)

            def col_to_tile(col, name, dtype=F32):
                pst = ps.tile([1, 64], F32, tag="ps")
                mm(pst[:], col[:], id64[:], is_transpose=True)
                row = sb.tile([1, 64], dtype, tag=name + "r")
                nc.scalar.copy(row[:], pst[:])
                t = cst.tile([128, 64], dtype, tag=name)
                nc.gpsimd.partition_broadcast(t[:], row[:])
                return t

            scale_t = col_to_tile(scalec, "scalet", BF16)
            shift_t = col_to_tile(shiftc, "shiftt", F32)
            for fi, ft in enumerate((scale_t, shift_t)):
                fjt = sb.tile([128, 1], F32, tag=f"fjt{fi}")
                nc.vector.tensor_copy(fjt[:], ft[:, 0:1])

            for w in range(NWIN):
                o1 = sb.tile([128, 64], F32, tag="o1")
                nc.vector.tensor_tensor(out=o1[:], in0=ybuf[:, w, :],
                                        in1=scale_t[:], op=OP.mult)
                o2 = sb.tile([128, 64], F32, tag="o2")
                nc.vector.tensor_tensor(out=o2[:], in0=o1[:], in1=shift_t[:],
                                        op=OP.add)
                nc.sync.dma_start(out=out_rows[w * 128:(w + 1) * 128, :],
                                  in_=o2[:])

        for _rep in range(int(os.environ.get('BASS_REPEAT', '1'))):
            body()

        for p in (psS, psU, psG, ps, ohsp, selp, grp, swp, relp, ohtp, tgp,
                  tbp, sb, cst):
            p.release()

    nc.finalize()
    return nc


def _host_prep(DKG):
    """Sort edges by head, shard by head range, build per-core slot arrays
    (index-only preprocessing)."""
    heads = np.asarray(DKG[:, 0], dtype=np.int64)
    tails = np.asarray(DKG[:, 1], dtype=np.int64)
    rels = np.asarray(DKG[:, 2], dtype=np.int64)

    order = np.argsort(heads, kind="stable")
    hs, ts, rs = heads[order], tails[order], rels[order]

    core_lo = np.searchsorted(hs, HPC * np.arange(NC), side="left")
    core_hi = np.searchsorted(hs, HPC * (np.arange(NC) + 1), side="left")

    winb = np.searchsorted(hs, WIN * np.arange(NC * NWIN), side="left")
    wine = np.searchsorted(hs, WIN * (np.arange(NC * NWIN) + 1), side="left")
    maxw = int((wine - winb).max())
    NBW = max(1, (maxw + 127) // 128)
    NBW = (NBW + GEXT - 1) // GEXT * GEXT   # multiple of the extraction chunk
    NB = NWIN * NBW
    S = NB * 128

    oht_dt = np.dtype(mybir.dt.np(OHT_DT))
    rel_dt = np.dtype(mybir.dt.np(REL_DT))
    per_core = []
    for c in range(NC):
        lo, hi = core_lo[c], core_hi[c]
        ch, ct, cr = hs[lo:hi], ts[lo:hi], rs[lo:hi]
        hrel = np.full(S, PAD_H, np.float32)
        rel = np.full(S, -1, np.int64)
        tail = np.zeros(S, np.int64)
        base = c * HPC
        for w in range(NWIN):
            wl = np.searchsorted(ch, base + w * WIN, side="left")
            wh = np.searchsorted(ch, base + (w + 1) * WIN, side="left")
            n = wh - wl
            o = w * NBW * 128
            hrel[o:o + n] = (ch[wl:wh] - base - w * WIN).astype(np.float32)
            rel[o:o + n] = cr[wl:wh]
            tail[o:o + n] = ct[wl:wh]
        hrelC = hrel.reshape(NB, 128).T.copy()
        t16 = tail.reshape(S // 16, 16).T.astype(np.int16)          # [16, S/16]
        t16r = np.tile(t16, (8, 1)).copy()                          # [128, S/16]
        # ohT [head k, blk b, edge m] = (hrel[b*128+m] == k), fp8
        hrel_bm = hrel.reshape(NB, 128)                             # [b, m]
        ohT = (np.arange(128, dtype=np.float32)[:, None, None]
               == hrel_bm[None, :, :]).astype(oht_dt)
        # relOH [edge p, blk b, r] = (rel[b*128+p] == r), fp8
        rel_bp = rel.reshape(NB, 128).T                             # [p, b]
        relOH = (rel_bp[:, :, None]
                 == np.arange(64, dtype=np.int64)[None, None, :]).astype(rel_dt)
        per_core.append(dict(hrelC=hrelC, tails16=t16r, ohT=ohT, relOH=relOH))
    return NBW, per_core


def prepare(X, DKG, drug_emb, rel_emb, tail_emb, W1, b1, W2, b2, gamma, beta,
            W3, W4, W5):
    f = np.float32
    bf = np.dtype(mybir.dt.np(BF16))
    NBW, per_core = _host_prep(np.asarray(DKG))
    nc = _build_nc(NBW)

    consts = dict(
        c_iota128b=np.broadcast_to(np.arange(128, dtype=f),
                                   (128, 128)).astype(bf),
        c_id128=np.eye(128, dtype=f),
        c_id64=np.eye(64, dtype=f),
        c_id128b=np.eye(128, dtype=f).astype(bf),
        c_ones64=np.ones((64, 1), f),
        c_ones128b=np.ones((128, 1), f).astype(bf),
    )
    tp = np.zeros((TROWS, D), f)
    tp[:N_TAIL] = np.asarray(tail_emb, f)
    weights = dict(
        tail_pad=tp,
        rel_emb=np.asarray(rel_emb, f),
        W1=np.asarray(W1, f), W2=np.asarray(W2, f), W3=np.asarray(W3, f),
        W4=np.asarray(W4, f), W5=np.asarray(W5, f),
        b1c=np.asarray(b1, f).reshape(D, 1),
        b2r=np.asarray(b2, f).reshape(1, D),
        gammac=np.asarray(gamma, f).reshape(D, 1),
        betac=np.asarray(beta, f).reshape(D, 1),
    )
    de = np.asarray(drug_emb, f)
    in_maps = []
    for c in range(NC):
        dr = np.zeros((DROWS, D), f)
        dr[:HPC] = de[c * HPC:(c + 1) * HPC]
        m = dict(weights)
        m.update(consts)
        m["drug_rows"] = dr
        pc = per_core[c]
        m["hrelC"] = pc["hrelC"]
        m["tails16"] = pc["tails16"]
        m["ohT"] = pc["ohT"]
        m["relOH"] = pc["relOH"]
        in_maps.append(m)
    return nc, in_maps


def kernel(X, DKG, drug_emb, rel_emb, tail_emb, W1, b1, W2, b2, gamma, beta,
           W3, W4, W5):
    X = np.asarray(X)
    nc, in_maps = prepare(X, DKG, drug_emb, rel_emb, tail_emb, W1, b1, W2, b2,
                          gamma, beta, W3, W4, W5)

    res = run_bass_kernel_spmd(nc, in_maps, core_ids=list(range(NC)))
    global LAST_RESULT
    LAST_RESULT = res
    out = np.concatenate([np.asarray(res.results[c]["out_rows"][:HPC], np.float32)
                          for c in range(NC)], axis=0)
    return out, X


LAST_RESULT = None


# revision 14
# speedup vs baseline: 2.0807x; 1.8920x over previous
"""GAT layer (gnn_message_passing) Trainium2 kernel — v2.

Math (after algebraic simplification of the reference):
  v3 = W3 @ W5[:64];  v4 = W4 @ W5[64:]           # [64]
  s3 = drug_emb @ v3                               # [N_DRUG]
  s4 = tail_emb @ v4                               # [N_TAIL]
  Sb = drug_emb @ (rel_emb * (W1 @ 1)).T + sum(b1) # [N_DRUG, N_REL]
  att_e  = leaky_relu(s3[h_e] + s4[t_e])
  p_e    = exp(att_e)            (softmax max-shift dropped: shift-invariant)
  w_e    = p_e * Sb[h_e, r_e]
  U[h]   = sum_e w_e * tail_emb[t_e];  den[h] = sum_e p_e
  neigh  = U / den
  y      = [drug_emb | neigh] @ W2 + b2;  out = batchnorm(y) (training stats)

Sharding: edges sorted by head on the host (index-only preprocessing);
8 cores own disjoint 2500-head ranges, so segment stats complete locally.
Only the 64x2 batchnorm statistics are all-reduced.

v2 device strategy (per core, per 128-head window, 128-edge blocks):
  - tailB DRAM table [20480, 128] bf16 rows [tail|s4|pad], built on device
    once; per-window dma_gather pulls 256B rows (s4 rides along).
  - host ships one-hot matrices as fp8: ohT (head one-hot, lhsT for the
    per-edge SW-row gather matmul) and relOH (rel one-hot for Sb column
    selection).
  - per block: 1 pG matmul (gather [Sb_row|s3] per edge), 1 fused
    tensor_scalar builds the wg-scaled scatter one-hot, 1 pU matmul
    accumulates [U|den] in PSUM.  den uses rhs col64 = 1/sv so that
    wg*(1/sv) = p.
  - per 4 blocks: one TT-mult + tensor_reduce extracts sv; one TT-add
    forms att.  Per 16 blocks: lrelu (on DVE), exp (ACT), reciprocal etc.
  - all edge-pass matmuls bf16/fp8 (single HW pass vs fp32's two).
"""

import os

import numpy as np

import concourse.bacc as bacc
import concourse.bass as bass
import concourse.tile as tile
from concourse import mybir
from concourse.bass_utils import run_bass_kernel_spmd

F32 = mybir.dt.float32
BF16 = mybir.dt.bfloat16
F8 = mybir.dt.float8e4
I16 = mybir.dt.int16
AF = mybir.ActivationFunctionType
OP = mybir.AluOpType

N_DRUG = 20000
N_TAIL = 20000
N_REL = 64
D = 64
NC = 8
HPC = N_DRUG // NC          # heads per core
WIN = 128                   # heads per window
NWIN = (HPC + WIN - 1) // WIN  # windows per core (20)
DROWS = NWIN * WIN          # padded drug rows per core (2560)
TROWS = 20480               # padded tail rows (160 chunks of 128)
TCH = 16                    # tail-table chunks per iteration
EPS = 1e-5
SLOPE = 0.01
GEXT = 4                    # blocks per extraction chunk (PSUM-bank bound)
GRP = 16                    # blocks per batched-scalar group
NQ = int(os.environ.get("K_NQ", "1"))      # SWDGE queues for dma_gather
GCH = int(os.environ.get("K_GCH", "8"))    # blocks per dma_gather call
SIMIDX = os.environ.get("K_SIMIDX", "0") == "1"  # full-128 idx AP (CoreSim)
DMA_SCRATCH = int(os.environ.get("K_SCRATCH", "16384"))
PAD_H = 999.0               # hrel sentinel for padded slots (no one-hot match)
F8OHT = os.environ.get("K_F8OHT", "0") == "1"   # ship ohT as fp8 (else bf16)
F8REL = os.environ.get("K_F8REL", "0") == "1"   # ship relOH as fp8 (else bf16)
OHT_DT = F8 if F8OHT else BF16
REL_DT = F8 if F8REL else BF16


def _build_nc(NBW: int):
    """Build the Bass module. NBW = 128-edge blocks per 128-head window."""
    NB = NWIN * NBW          # blocks per core
    S = NB * 128             # edge slots per core

    nc = bacc.Bacc(None, num_devices=NC, num_swdge_queues=NQ,
                   dynamic_dma_scratch_size=DMA_SCRATCH)

    # ---- I/O ----
    def inp(name, shape, dtype=F32):
        return nc.declare_dram_parameter(name, list(shape), dtype, isOutput=False)

    tail_pad = inp("tail_pad", (TROWS, D))
    drug_rows = inp("drug_rows", (DROWS, D))
    rel_emb = inp("rel_emb", (N_REL, D))
    W1 = inp("W1", (D, D))
    W2 = inp("W2", (2 * D, D))
    W3 = inp("W3", (D, D))
    W4 = inp("W4", (D, D))
    W5 = inp("W5", (2 * D, 1))
    b1c = inp("b1c", (D, 1))
    b2r = inp("b2r", (1, D))
    gammac = inp("gammac", (D, 1))
    betac = inp("betac", (D, 1))

    hrelC = inp("hrelC", (128, NB))          # hrel per slot (pads = PAD_H)
    tails16 = inp("tails16", (128, S // 16), I16)
    ohT_in = inp("ohT", (128, NB, 128), OHT_DT)  # head one-hot [head, blk, edge]
    relOH_in = inp("relOH", (128, NB, 64), REL_DT)  # rel one-hot [edge, blk, rel]

    c_iota128b = inp("c_iota128b", (128, 128), BF16)
    c_id128 = inp("c_id128", (128, 128))
    c_id64 = inp("c_id64", (64, 64))
    c_id128b = inp("c_id128b", (128, 128), BF16)
    c_ones64 = inp("c_ones64", (64, 1))
    c_ones128b = inp("c_ones128b", (128, 1), BF16)

    out_rows = nc.declare_dram_parameter("out_rows", [DROWS, D], F32, isOutput=True)

    # DRAM scratch: bf16 tail table rows [tail(64) | s4 | junk(63)]
    tailB = nc.dram_tensor("tailB", [TROWS, 128], BF16)

    # collective bounce buffers
    cc_in = nc.dram_tensor("cc_in", [D, 2], F32)
    cc_out = nc.dram_tensor("cc_out", [D, 2], F32, addr_space="Shared")

    with tile.TileContext(nc) as tc:
        cst = tc.alloc_tile_pool(name="cst", bufs=1)
        sb = tc.alloc_tile_pool(name="sb", bufs=2)
        tbp = tc.alloc_tile_pool(name="tbp", bufs=2)
        tgp = tc.alloc_tile_pool(name="tgp", bufs=2)
        ohtp = tc.alloc_tile_pool(name="ohtp", bufs=2)
        relp = tc.alloc_tile_pool(name="relp", bufs=2)
        swp = tc.alloc_tile_pool(name="swp", bufs=2)
        grp = tc.alloc_tile_pool(name="grp", bufs=2)
        selp = tc.alloc_tile_pool(name="selp", bufs=2)
        ohsp = tc.alloc_tile_pool(name="ohsp", bufs=4)
        wtp = tc.alloc_tile_pool(name="wtp", bufs=4)
        ps = tc.alloc_tile_pool(name="ps", bufs=1, space="PSUM")
        psG = tc.alloc_tile_pool(name="psG", bufs=2, space="PSUM")
        psU = tc.alloc_tile_pool(name="psU", bufs=2, space="PSUM")
        psS = tc.alloc_tile_pool(name="psS", bufs=1, space="PSUM")

        def body():
            # absorber: first DVE instruction after the init barrier must
            # carry no data wait.
            dve0 = cst.tile([128, 1], F32, tag="dve0")
            nc.vector.memset(dve0[:], 0.0)

            def mm(out, lhsT, rhs, start=True, stop=True, is_transpose=None):
                return nc.tensor.matmul(out, lhsT, rhs, start=start, stop=stop,
                                        is_transpose=is_transpose,
                                        skip_group_check=True)

            import bass_rust as _br

            def dep(a, b):
                _br.add_dep_helper(a.ins, b.ins, sync=True, reason="wait-routing")

            def load(pool, src_ap, shape, dtype=F32, name=None):
                t = pool.tile(list(shape), dtype, tag=name)
                nc.sync.dma_start(out=t[:], in_=src_ap)
                return t

            # ---- constants into SBUF ----
            iota128b = load(cst, c_iota128b[:, :], (128, 128), BF16, "iota128b")
            id128 = load(cst, c_id128[:, :], (128, 128), name="id128")
            id64 = load(cst, c_id64[:, :], (64, 64), name="id64")
            id128b = load(cst, c_id128b[:, :], (128, 128), BF16, "id128b")
            ones64 = load(cst, c_ones64[:, :], (64, 1), name="ones64")
            ones128b = load(cst, c_ones128b[:, :], (128, 1), BF16, "ones128b")

            hrelC_t = load(cst, hrelC[:, :], (128, NB), name="hrelC")
            tails_t = load(cst, tails16[:, :], (128, S // 16), I16, name="tails")

            w1t = load(cst, W1[:, :], (64, 64), name="w1")
            w2a = load(cst, W2[0:64, :], (64, 64), name="w2a")
            w2b = load(cst, W2[64:128, :], (64, 64), name="w2b")
            w3t = load(cst, W3[:, :], (64, 64), name="w3")
            w4t = load(cst, W4[:, :], (64, 64), name="w4")
            w5a = load(cst, W5[0:64, :], (64, 1), name="w5a")
            w5b = load(cst, W5[64:128, :], (64, 1), name="w5b")
            relt = load(cst, rel_emb[:, :], (64, 64), name="relt")
            b1col = load(cst, b1c[:, :], (64, 1), name="b1col")
            b2row = load(cst, b2r[:, :], (1, 64), name="b2row")
            gcol = load(cst, gammac[:, :], (64, 1), name="gcol")
            bcol = load(cst, betac[:, :], (64, 1), name="bcol")

            # ---- phase 1: weight folding (fp32, tiny) ----
            def transpose_to(pool, src_t, k, m, name):
                pst = ps.tile([m, k], F32, tag="ps")
                ident = id64 if k == 64 else id128
                mm(pst[:], src_t[:], ident[:, 0:k], is_transpose=True)
                dst = pool.tile([m, k], F32, tag=name)
                nc.scalar.copy(dst[:], pst[:])
                return dst

            w3T = transpose_to(cst, w3t, 64, 64, "w3T")
            w4T = transpose_to(cst, w4t, 64, 64, "w4T")
            w1T = transpose_to(cst, w1t, 64, 64, "w1T")
            relT = transpose_to(cst, relt, 64, 64, "relT")

            def mm_to_sbuf(pool, lhsT, rhs, m, n, name, dtype=F32):
                pst = ps.tile([m, n], F32, tag="ps")
                mm(pst[:], lhsT, rhs)
                dst = pool.tile([m, n], dtype, tag=name)
                nc.scalar.copy(dst[:], pst[:])
                return dst

            v3b = mm_to_sbuf(cst, w3T[:], w5a[:], 64, 1, "v3b", BF16)   # [64,1]
            v4 = mm_to_sbuf(cst, w4T[:], w5b[:], 64, 1, "v4")           # [64,1]
            w1s = mm_to_sbuf(cst, w1T[:], ones64[:], 64, 1, "w1s")      # [64,1]
            b1s = mm_to_sbuf(cst, b1col[:], ones64[:], 1, 1, "b1s")     # [1,1]

            # v4 broadcast tile [128, 64] fp32 (for the tail-table build)
            psv4r = ps.tile([1, 64], F32, tag="ps")
            mm(psv4r[:], v4[:], id64[:], is_transpose=True)
            v4row = cst.tile([1, 64], F32, tag="v4row")
            nc.scalar.copy(v4row[:], psv4r[:])
            v4tile = cst.tile([128, 64], F32, tag="v4tile")
            nc.gpsimd.partition_broadcast(v4tile[:], v4row[:])

            b1s_tile = cst.tile([128, 1], F32, tag="b1stile")
            nc.gpsimd.partition_broadcast(b1s_tile[:], b1s[:])

            b2tile = cst.tile([128, 64], F32, tag="b2tile")
            nc.gpsimd.partition_broadcast(b2tile[:], b2row[:])

            # bf16 weights for the edge/output matmuls
            w2ab = cst.tile([64, 64], BF16, tag="w2ab")
            nc.scalar.copy(w2ab[:], w2a[:])
            w2bb = cst.tile([64, 64], BF16, tag="w2bb")
            nc.scalar.copy(w2bb[:], w2b[:])

            # M_T = rel_emb.T * w1s (per-partition scale), bf16
            MTb = cst.tile([64, 64], BF16, tag="MTb")
            nc.vector.tensor_scalar(MTb[:], relT[:], w1s[:], None, OP.mult)

            # DVE fences: absorb preload DMA waits so hot-loop compact DVE
            # ops never need more than one embedded wait.
            for fi, ft in enumerate((iota128b, hrelC_t, v4tile, b2tile,
                                     gcol, bcol)):
                np_ = ft.shape[0]
                fj = cst.tile([np_, 1], F32, tag=f"fj{fi}")
                nc.vector.tensor_copy(fj[:], ft[0:np_, 0:1])
            fj16 = cst.tile([128, 1], I16, tag="fj16")
            nc.vector.tensor_copy(fj16[:], tails_t[:, 0:1])

            # ---- phase 1b: tail table build ----
            NIT = TROWS // (TCH * 128)
            for it in range(NIT):
                base = it * TCH * 128
                tch = tbp.tile([128, TCH, 64], F32, tag="tch")
                src = tail_pad[base:base + TCH * 128, :].rearrange(
                    "(c p) d -> p c d", p=128)
                nc.sync.dma_start(out=tch[:], in_=src)
                tbb = tbp.tile([128, TCH, 128], BF16, tag="tbb")
                nc.vector.memset(tbb[:, :, 65:128], 0.0)
                s4f = tbp.tile([128, TCH], F32, tag="s4f")
                for c in range(TCH):
                    junk = sb.tile([128, 64], F32, tag="junk")
                    nc.vector.scalar_tensor_tensor(
                        out=junk[:], in0=tch[:, c, :], scalar=0.0,
                        in1=v4tile[:], op0=OP.bypass, op1=OP.mult,
                        accum_out=s4f[:, c:c + 1])
                nc.vector.tensor_copy(tbb[:, :, 0:64], tch[:])
                nc.vector.tensor_copy(tbb[:, :, 64], s4f[:])
                dst = tailB[base:base + TCH * 128, :].rearrange(
                    "(c p) d -> p c d", p=128)
                nc.sync.dma_start(out=dst, in_=tbb[:])

            # ---- phase 2+3 fused: per-window pipeline ----
            ybuf = cst.tile([128, NWIN, 64], BF16, tag="ybuf")
            pStat = psS.tile([64, 2], F32, tag="pStat")
            drugTs = []

            NCH = (NBW + GCH - 1) // GCH
            for w in range(NWIN):
                # window drug prep -> SW = [Sb | s3] bf16 [128, 65]
                dchunk = sb.tile([128, 64], F32, tag="dchunk")
                nc.sync.dma_start(out=dchunk[:],
                                  in_=drug_rows[w * 128:(w + 1) * 128, :])
                psDT = ps.tile([64, 128], F32, tag="ps")
                mm(psDT[:], dchunk[:], id128[:], is_transpose=True)
                dTb = cst.tile([64, 128], BF16, tag=f"drugT{w}")
                nc.scalar.copy(dTb[:], psDT[:])
                drugTs.append(dTb)
                SW = swp.tile([128, 65], BF16, tag="SW")
                psSb = ps.tile([128, 64], F32, tag="ps")
                mm(psSb[:], dTb[:], MTb[:])
                nc.scalar.activation(SW[:, 0:64], psSb[:], AF.Identity,
                                     bias=b1s_tile[:], scale=1.0)
                psS3 = ps.tile([128, 1], F32, tag="ps")
                mm(psS3[:], dTb[:], v3b[:])
                nc.scalar.copy(SW[:, 64:65], psS3[:])

                # gather this window's tail rows [128, NBW, 128] bf16
                tg = tgp.tile([128, NBW, 128], BF16, tag="tg")
                gat_i = None
                for g0 in range(0, NBW, GCH):
                    gn = min(GCH, NBW - g0) * 128
                    io = (w * NBW + g0) * 8
                    gat_i = nc.gpsimd.dma_gather(
                        out_ap=tg[:, g0:g0 + gn // 128, :],
                        in_ap=tailB[:, :],
                        idxs_ap=(tails_t[:, io:io + gn // 16] if SIMIDX
                                 else tails_t[0:16, io:io + gn // 16]),
                        num_idxs=gn,
                        num_idxs_reg=gn,
                        elem_size=128,
                        queue_num=(w * NCH + g0 // GCH) % NQ,
                    )

                # stream this window's one-hots
                ohTw = ohtp.tile([128, NBW, 128], OHT_DT, tag="ohTw")
                nc.sync.dma_start(out=ohTw[:],
                                  in_=ohT_in[:, w * NBW:(w + 1) * NBW, :])
                relw = relp.tile([128, NBW, 64], REL_DT, tag="relw")
                nc.sync.dma_start(out=relw[:],
                                  in_=relOH_in[:, w * NBW:(w + 1) * NBW, :])

                pU = psU.tile([128, 65], F32, tag="pU")

                for j0 in range(0, NBW, GRP):
                    g = min(GRP, NBW - j0)
                    sv16 = grp.tile([128, GRP], F32, tag="sv16")
                    att16 = grp.tile([128, GRP], F32, tag="att16")
                    s4c16 = grp.tile([128, GRP], F32, tag="s4c16")
                    # s4 junction copy (absorbs the gather DMA wait)
                    s4i = nc.scalar.copy(s4c16[:, 0:g], tg[:, j0:j0 + g, 64])
                    dep(s4i, gat_i)
                    for q in range(0, g, GEXT):
                        psG4 = psG.tile([128, GEXT, 65], F32, tag="psG4")
                        for i in range(GEXT):
                            j = j0 + q + i
                            mm(psG4[:, i, :], ohTw[:, j, :], SW[:],
                               start=True, stop=True)
                        sel4 = selp.tile([128, GEXT, 64], BF16, tag="sel4")
                        nc.vector.tensor_tensor(
                            out=sel4[:], in0=psG4[:, :, 0:64],
                            in1=relw[:, j0 + q:j0 + q + GEXT, :], op=OP.mult)
                        nc.vector.tensor_reduce(
                            out=sv16[:, q:q + GEXT], in_=sel4[:],
                            axis=mybir.AxisListType.X, op=OP.add)
                        nc.vector.tensor_tensor(
                            out=att16[:, q:q + GEXT], in0=psG4[:, :, 64],
                            in1=s4c16[:, q:q + GEXT], op=OP.add)
                    # group scalar pipeline [128, g]
                    l16 = grp.tile([128, GRP], F32, tag="l16")
                    nc.vector.scalar_tensor_tensor(
                        out=l16[:, 0:g], in0=att16[:, 0:g], scalar=SLOPE,
                        in1=att16[:, 0:g], op0=OP.mult, op1=OP.max)
                    p16 = grp.tile([128, GRP], F32, tag="p16")
                    nc.scalar.activation(p16[:, 0:g], l16[:, 0:g], AF.Exp)
                    svc16 = grp.tile([128, GRP], F32, tag="svc16")
                    nc.vector.tensor_scalar(svc16[:, 0:g], sv16[:, 0:g], 1e-12,
                                            None, OP.add)
                    wg16 = grp.tile([128, GRP], F32, tag="wg16")
                    nc.vector.tensor_tensor(out=wg16[:, 0:g], in0=p16[:, 0:g],
                                            in1=svc16[:, 0:g], op=OP.mult)
                    rc16 = grp.tile([128, GRP], F32, tag="rc16")
                    nc.vector.reciprocal(rc16[:, 0:g], svc16[:, 0:g])
                    nc.scalar.copy(tg[:, j0:j0 + g, 64], rc16[:, 0:g])
                    for jj in range(g):
                        j = j0 + jj
                        ohS = ohsp.tile([128, 128], BF16, tag="ohS")
                        nc.vector.tensor_scalar(
                            ohS[:], iota128b[:],
                            hrelC_t[:, w * NBW + j:w * NBW + j + 1],
                            None, OP.is_equal)
                        wt = wtp.tile([128, 65], BF16, tag="wt")
                        nc.vector.tensor_scalar(
                            wt[:], tg[:, j, 0:65], wg16[:, jj:jj + 1],
                            None, OP.mult)
                        mm(pU[:], ohS[:], wt[:],
                           start=(j == 0), stop=(j == NBW - 1))

                # window reduction -> neigh -> y -> stats
                dsafe = sb.tile([128, 1], F32, tag="dsafe")
                nc.vector.tensor_scalar(dsafe[:], pU[:, 64:65], 1e-30, None,
                                        OP.add)
                recip = sb.tile([128, 1], F32, tag="recip")
                nc.vector.reciprocal(recip[:], dsafe[:])
                nw = sb.tile([128, 64], F32, tag="nw")
                nc.vector.tensor_scalar(nw[:], pU[:, 0:64], recip[:], None,
                                        OP.mult)
                psNT = ps.tile([64, 128], F32, tag="psT")
                mm(psNT[:], nw[:], id128[:], is_transpose=True)
                nT = sb.tile([64, 128], BF16, tag="nT")
                nc.scalar.copy(nT[:], psNT[:])
                pY = ps.tile([128, 64], F32, tag="ps")
                mm(pY[:], drugTs[w][:], w2ab[:], start=True, stop=False)
                mm(pY[:], nT[:], w2bb[:], start=False, stop=True)
                nc.vector.tensor_tensor(out=ybuf[:, w, :], in0=pY[:],
                                        in1=b2tile[:], op=OP.add)
                sq = sb.tile([128, 64], BF16, tag="sq")
                nc.vector.tensor_tensor(out=sq[:], in0=ybuf[:, w, :],
                                        in1=ybuf[:, w, :], op=OP.mult)
                mm(pStat[:, 0:1], ybuf[:, w, :], ones128b[:],
                   start=(w == 0), stop=(w == NWIN - 1))
                mm(pStat[:, 1:2], sq[:], ones128b[:],
                   start=(w == 0), stop=(w == NWIN - 1))

            # ---- phase 4: batchnorm ----
            statsb = sb.tile([64, 2], F32, tag="statsb")
            nc.scalar.copy(statsb[:], pStat[:])
            nc.sync.dma_start(out=cc_in[:, :], in_=statsb[:])
            nc.gpsimd.collective_compute(
                "AllReduce", OP.add, replica_groups=[list(range(NC))],
                ins=[cc_in[:, :]], outs=[cc_out[:, :]])
            statsg = sb.tile([64, 2], F32, tag="statsg")
            nc.sync.dma_start(out=statsg[:], in_=cc_out[:, :])
            fjs = sb.tile([64, 1], F32, tag="fjs")
            nc.vector.tensor_copy(fjs[:], statsg[:, 0:1])

            mean = sb.tile([64, 1], F32, tag="mean")
            nc.vector.tensor_scalar(mean[:], statsg[:, 0:1], 1.0 / N_DRUG,
                                    None, OP.mult)
            ex2 = sb.tile([64, 1], F32, tag="ex2")
            nc.vector.tensor_scalar(ex2[:], statsg[:, 1:2], 1.0 / N_DRUG,
                                    None, OP.mult)
            msq = sb.tile([64, 1], F32, tag="msq")
            nc.vector.tensor_tensor(out=msq[:], in0=mean[:], in1=mean[:],
                                    op=OP.mult)
            var = sb.tile([64, 1], F32, tag="var")
            nc.vector.tensor_tensor(out=var[:], in0=ex2[:], in1=msq[:],
                                    op=OP.subtract)
            vare = sb.tile([64, 1], F32, tag="vare")
            nc.vector.tensor_scalar(vare[:], var[:], EPS, None, OP.add)
            sd = sb.tile([64, 1], F32, tag="sd")
            nc.scalar.activation(sd[:], vare[:], AF.Sqrt)
            rstd = sb.tile([64, 1], F32, tag="rstd")
            nc.vector.reciprocal(rstd[:], sd[:])
            scalec = sb.tile([64, 1], F32, tag="scalec")
            nc.vector.tensor_tensor(out=scalec[:], in0=gcol[:], in1=rstd[:],
                                    op=OP.mult)
            tmp = sb.tile([64, 1], F32, tag="tmp")
            nc.vector.tensor_tensor(out=tmp[:], in0=mean[:], in1=scalec[:],
                                    op=OP.mult)
            shiftc = sb.tile([64, 1], F32, tag="shiftc")
            nc.vector.tensor_tensor(out=shiftc[:], in0=bcol[:], in1=tmp[:],
                                    op=OP.subtract)

            def col_to_tile(col, name, dtype=F32):
                pst = ps.tile([1, 64], F32, tag="ps")
                mm(pst[:], col[:], id64[:], is_transpose=True)
                row = sb.tile([1, 64], dtype, tag=name + "r")
                nc.scalar.copy(row[:], pst[:])
                t = cst.tile([128, 64], dtype, tag=name)
                nc.gpsimd.partition_broadcast(t[:], row[:])
                return t

            scale_t = col_to_tile(scalec, "scalet", BF16)
            shift_t = col_to_tile(shiftc, "shiftt", F32)
            for fi, ft in enumerate((scale_t, shift_t)):
                fjt = sb.tile([128, 1], F32, tag=f"fjt{fi}")
                nc.vector.tensor_copy(fjt[:], ft[:, 0:1])

            for w in range(NWIN):
                o1 = sb.tile([128, 64], F32, tag="o1")
                nc.vector.tensor_tensor(out=o1[:], in0=ybuf[:, w, :],
                                        in1=scale_t[:], op=OP.mult)
                o2 = sb.tile([128, 64], F32, tag="o2")
                nc.vector.tensor_tensor(out=o2[:], in0=o1[:], in1=shift_t[:],
                                        op=OP.add)
                nc.sync.dma_start(out=out_rows[w * 128:(w + 1) * 128, :],
                                  in_=o2[:])

        for _rep in range(int(os.environ.get('BASS_REPEAT', '1'))):
            body()

        for p in (psS, psU, psG, ps, wtp, ohsp, selp, grp, swp, relp, ohtp,
                  tgp, tbp, sb, cst):
            p.release()

    nc.finalize()
    return nc


def _host_prep(DKG):
    """Sort edges by head, shard by head range, build per-core slot arrays
    (index-only preprocessing)."""
    heads = np.asarray(DKG[:, 0], dtype=np.int64)
    tails = np.asarray(DKG[:, 1], dtype=np.int64)
    rels = np.asarray(DKG[:, 2], dtype=np.int64)

    order = np.argsort(heads, kind="stable")
    hs, ts, rs = heads[order], tails[order], rels[order]

    core_lo = np.searchsorted(hs, HPC * np.arange(NC), side="left")
    core_hi = np.searchsorted(hs, HPC * (np.arange(NC) + 1), side="left")

    winb = np.searchsorted(hs, WIN * np.arange(NC * NWIN), side="left")
    wine = np.searchsorted(hs, WIN * (np.arange(NC * NWIN) + 1), side="left")
    maxw = int((wine - winb).max())
    NBW = max(1, (maxw + 127) // 128)
    NBW = (NBW + GEXT - 1) // GEXT * GEXT   # multiple of the extraction chunk
    NB = NWIN * NBW
    S = NB * 128

    oht_dt = np.dtype(mybir.dt.np(OHT_DT))
    rel_dt = np.dtype(mybir.dt.np(REL_DT))
    per_core = []
    for c in range(NC):
        lo, hi = core_lo[c], core_hi[c]
        ch, ct, cr = hs[lo:hi], ts[lo:hi], rs[lo:hi]
        hrel = np.full(S, PAD_H, np.float32)
        rel = np.full(S, -1, np.int64)
        tail = np.zeros(S, np.int64)
        base = c * HPC
        for w in range(NWIN):
            wl = np.searchsorted(ch, base + w * WIN, side="left")
            wh = np.searchsorted(ch, base + (w + 1) * WIN, side="left")
            n = wh - wl
            o = w * NBW * 128
            hrel[o:o + n] = (ch[wl:wh] - base - w * WIN).astype(np.float32)
            rel[o:o + n] = cr[wl:wh]
            tail[o:o + n] = ct[wl:wh]
        hrelC = hrel.reshape(NB, 128).T.copy()
        t16 = tail.reshape(S // 16, 16).T.astype(np.int16)          # [16, S/16]
        t16r = np.tile(t16, (8, 1)).copy()                          # [128, S/16]
        # ohT [head k, blk b, edge m] = (hrel[b*128+m] == k), fp8
        hrel_bm = hrel.reshape(NB, 128)                             # [b, m]
        ohT = (np.arange(128, dtype=np.float32)[:, None, None]
               == hrel_bm[None, :, :]).astype(oht_dt)
        # relOH [edge p, blk b, r] = (rel[b*128+p] == r), fp8
        rel_bp = rel.reshape(NB, 128).T                             # [p, b]
        relOH = (rel_bp[:, :, None]
                 == np.arange(64, dtype=np.int64)[None, None, :]).astype(rel_dt)
        per_core.append(dict(hrelC=hrelC, tails16=t16r, ohT=ohT, relOH=relOH))
    return NBW, per_core


def prepare(X, DKG, drug_emb, rel_emb, tail_emb, W1, b1, W2, b2, gamma, beta,
            W3, W4, W5):
    f = np.float32
    bf = np.dtype(mybir.dt.np(BF16))
    NBW, per_core = _host_prep(np.asarray(DKG))
    nc = _build_nc(NBW)

    consts = dict(
        c_iota128b=np.broadcast_to(np.arange(128, dtype=f),
                                   (128, 128)).astype(bf),
        c_id128=np.eye(128, dtype=f),
        c_id64=np.eye(64, dtype=f),
        c_id128b=np.eye(128, dtype=f).astype(bf),
        c_ones64=np.ones((64, 1), f),
        c_ones128b=np.ones((128, 1), f).astype(bf),
    )
    tp = np.zeros((TROWS, D), f)
    tp[:N_TAIL] = np.asarray(tail_emb, f)
    weights = dict(
        tail_pad=tp,
        rel_emb=np.asarray(rel_emb, f),
        W1=np.asarray(W1, f), W2=np.asarray(W2, f), W3=np.asarray(W3, f),
        W4=np.asarray(W4, f), W5=np.asarray(W5, f),
        b1c=np.asarray(b1, f).reshape(D, 1),
        b2r=np.asarray(b2, f).reshape(1, D),
        gammac=np.asarray(gamma, f).reshape(D, 1),
        betac=np.asarray(beta, f).reshape(D, 1),
    )
    de = np.asarray(drug_emb, f)
    in_maps = []
    for c in range(NC):
        dr = np.zeros((DROWS, D), f)
        dr[:HPC] = de[c * HPC:(c + 1) * HPC]
        m = dict(weights)
        m.update(consts)
        m["drug_rows"] = dr
        pc = per_core[c]
        m["hrelC"] = pc["hrelC"]
        m["tails16"] = pc["tails16"]
        m["ohT"] = pc["ohT"]
        m["relOH"] = pc["relOH"]
        in_maps.append(m)
    return nc, in_maps


def kernel(X, DKG, drug_emb, rel_emb, tail_emb, W1, b1, W2, b2, gamma, beta,
           W3, W4, W5):
    X = np.asarray(X)
    nc, in_maps = prepare(X, DKG, drug_emb, rel_emb, tail_emb, W1, b1, W2, b2,
                          gamma, beta, W3, W4, W5)

    res = run_bass_kernel_spmd(nc, in_maps, core_ids=list(range(NC)))
    global LAST_RESULT
    LAST_RESULT = res
    out = np.concatenate([np.asarray(res.results[c]["out_rows"][:HPC], np.float32)
                          for c in range(NC)], axis=0)
    return out, X


LAST_RESULT = None


# revision 15
# speedup vs baseline: 2.3488x; 1.1289x over previous
"""GAT layer (gnn_message_passing) Trainium2 kernel — v2.

Math (after algebraic simplification of the reference):
  v3 = W3 @ W5[:64];  v4 = W4 @ W5[64:]           # [64]
  s3 = drug_emb @ v3                               # [N_DRUG]
  s4 = tail_emb @ v4                               # [N_TAIL]
  Sb = drug_emb @ (rel_emb * (W1 @ 1)).T + sum(b1) # [N_DRUG, N_REL]
  att_e  = leaky_relu(s3[h_e] + s4[t_e])
  p_e    = exp(att_e)            (softmax max-shift dropped: shift-invariant)
  w_e    = p_e * Sb[h_e, r_e]
  U[h]   = sum_e w_e * tail_emb[t_e];  den[h] = sum_e p_e
  neigh  = U / den
  y      = [drug_emb | neigh] @ W2 + b2;  out = batchnorm(y) (training stats)

Sharding: edges sorted by head on the host (index-only preprocessing);
8 cores own disjoint 2500-head ranges, so segment stats complete locally.
Only the 64x2 batchnorm statistics are all-reduced.

v2 device strategy (per core, per 128-head window, 128-edge blocks):
  - tailB DRAM table [20480, 128] bf16 rows [tail|s4|pad], built on device
    once; per-window dma_gather pulls 256B rows (s4 rides along).
  - host ships one-hot matrices as fp8: ohT (head one-hot, lhsT for the
    per-edge SW-row gather matmul) and relOH (rel one-hot for Sb column
    selection).
  - per block: 1 pG matmul (gather [Sb_row|s3] per edge), 1 fused
    tensor_scalar builds the wg-scaled scatter one-hot, 1 pU matmul
    accumulates [U|den] in PSUM.  den uses rhs col64 = 1/sv so that
    wg*(1/sv) = p.
  - per 4 blocks: one TT-mult + tensor_reduce extracts sv; one TT-add
    forms att.  Per 16 blocks: lrelu (on DVE), exp (ACT), reciprocal etc.
  - all edge-pass matmuls bf16/fp8 (single HW pass vs fp32's two).
"""

import os

import numpy as np

import concourse.bacc as bacc
import concourse.bass as bass
import concourse.tile as tile
from concourse import mybir
from concourse.bass_utils import run_bass_kernel_spmd

F32 = mybir.dt.float32
BF16 = mybir.dt.bfloat16
F8 = mybir.dt.float8e4
I16 = mybir.dt.int16
AF = mybir.ActivationFunctionType
OP = mybir.AluOpType

N_DRUG = 20000
N_TAIL = 20000
N_REL = 64
D = 64
NC = 8
HPC = N_DRUG // NC          # heads per core
WIN = 128                   # heads per window
NWIN = (HPC + WIN - 1) // WIN  # windows per core (20)
DROWS = NWIN * WIN          # padded drug rows per core (2560)
TROWS = 20480               # padded tail rows (160 chunks of 128)
TCH = 16                    # tail-table chunks per iteration
EPS = 1e-5
SLOPE = 0.01
GEXT = 4                    # blocks per extraction chunk (PSUM-bank bound)
GRP = 16                    # blocks per batched-scalar group
NQ = int(os.environ.get("K_NQ", "1"))      # SWDGE queues for dma_gather
GCH = int(os.environ.get("K_GCH", "8"))    # blocks per dma_gather call
SIMIDX = os.environ.get("K_SIMIDX", "0") == "1"  # full-128 idx AP (CoreSim)
DMA_SCRATCH = int(os.environ.get("K_SCRATCH", "16384"))
PAD_H = 999.0               # hrel sentinel for padded slots (no one-hot match)
F8OHT = os.environ.get("K_F8OHT", "0") == "1"   # ship ohT as fp8 (else bf16)
F8REL = os.environ.get("K_F8REL", "0") == "1"   # ship relOH as fp8 (else bf16)
OHT_DT = F8 if F8OHT else BF16
REL_DT = F8 if F8REL else BF16


def _build_nc(NBW: int):
    """Build the Bass module. NBW = 128-edge blocks per 128-head window."""
    NB = NWIN * NBW          # blocks per core
    S = NB * 128             # edge slots per core

    nc = bacc.Bacc(None, num_devices=NC, num_swdge_queues=NQ,
                   dynamic_dma_scratch_size=DMA_SCRATCH)

    # ---- I/O ----
    def inp(name, shape, dtype=F32):
        return nc.declare_dram_parameter(name, list(shape), dtype, isOutput=False)

    tail_pad = inp("tail_pad", (TROWS, D))
    drug_rows = inp("drug_rows", (DROWS, D))
    rel_emb = inp("rel_emb", (N_REL, D))
    W1 = inp("W1", (D, D))
    W2 = inp("W2", (2 * D, D))
    W3 = inp("W3", (D, D))
    W4 = inp("W4", (D, D))
    W5 = inp("W5", (2 * D, 1))
    b1c = inp("b1c", (D, 1))
    b2r = inp("b2r", (1, D))
    gammac = inp("gammac", (D, 1))
    betac = inp("betac", (D, 1))

    hrelC = inp("hrelC", (128, NB))          # hrel per slot (pads = PAD_H)
    tails16 = inp("tails16", (128, S // 16), I16)
    ohT_in = inp("ohT", (128, NB, 128), OHT_DT)  # head one-hot [head, blk, edge]
    relOH_in = inp("relOH", (128, NB, 64), REL_DT)  # rel one-hot [edge, blk, rel]
    ohE_in = inp("ohE", (128, NB, 128), OHT_DT)  # scatter one-hot [edge, blk, head]

    c_iota128b = inp("c_iota128b", (128, 128), BF16)
    c_id128 = inp("c_id128", (128, 128))
    c_id64 = inp("c_id64", (64, 64))
    c_id128b = inp("c_id128b", (128, 128), BF16)
    c_ones64 = inp("c_ones64", (64, 1))
    c_ones128b = inp("c_ones128b", (128, 1), BF16)

    out_rows = nc.declare_dram_parameter("out_rows", [DROWS, D], F32, isOutput=True)

    # DRAM scratch: bf16 tail table rows [tail(64) | s4 | junk(63)]
    tailB = nc.dram_tensor("tailB", [TROWS, 128], BF16)

    # collective bounce buffers
    cc_in = nc.dram_tensor("cc_in", [D, 2], F32)
    cc_out = nc.dram_tensor("cc_out", [D, 2], F32, addr_space="Shared")

    with tile.TileContext(nc) as tc:
        cst = tc.alloc_tile_pool(name="cst", bufs=1)
        sb = tc.alloc_tile_pool(name="sb", bufs=2)
        tbp = tc.alloc_tile_pool(name="tbp", bufs=2)
        tgp = tc.alloc_tile_pool(name="tgp", bufs=2)
        ohtp = tc.alloc_tile_pool(name="ohtp", bufs=2)
        relp = tc.alloc_tile_pool(name="relp", bufs=2)
        swp = tc.alloc_tile_pool(name="swp", bufs=2)
        grp = tc.alloc_tile_pool(name="grp", bufs=2)
        selp = tc.alloc_tile_pool(name="selp", bufs=2)
        ohep = tc.alloc_tile_pool(name="ohep", bufs=2)
        wtp = tc.alloc_tile_pool(name="wtp", bufs=4)
        ps = tc.alloc_tile_pool(name="ps", bufs=1, space="PSUM")
        psG = tc.alloc_tile_pool(name="psG", bufs=2, space="PSUM")
        psU = tc.alloc_tile_pool(name="psU", bufs=2, space="PSUM")
        psS = tc.alloc_tile_pool(name="psS", bufs=1, space="PSUM")

        def body():
            # absorber: first DVE instruction after the init barrier must
            # carry no data wait.
            dve0 = cst.tile([128, 1], F32, tag="dve0")
            nc.vector.memset(dve0[:], 0.0)

            def mm(out, lhsT, rhs, start=True, stop=True, is_transpose=None):
                return nc.tensor.matmul(out, lhsT, rhs, start=start, stop=stop,
                                        is_transpose=is_transpose,
                                        skip_group_check=True)

            import bass_rust as _br

            def dep(a, b):
                _br.add_dep_helper(a.ins, b.ins, sync=True, reason="wait-routing")

            def load(pool, src_ap, shape, dtype=F32, name=None):
                t = pool.tile(list(shape), dtype, tag=name)
                nc.sync.dma_start(out=t[:], in_=src_ap)
                return t

            # ---- constants into SBUF ----
            iota128b = load(cst, c_iota128b[:, :], (128, 128), BF16, "iota128b")
            id128 = load(cst, c_id128[:, :], (128, 128), name="id128")
            id64 = load(cst, c_id64[:, :], (64, 64), name="id64")
            id128b = load(cst, c_id128b[:, :], (128, 128), BF16, "id128b")
            ones64 = load(cst, c_ones64[:, :], (64, 1), name="ones64")
            ones128b = load(cst, c_ones128b[:, :], (128, 1), BF16, "ones128b")

            hrelC_t = load(cst, hrelC[:, :], (128, NB), name="hrelC")
            tails_t = load(cst, tails16[:, :], (128, S // 16), I16, name="tails")

            w1t = load(cst, W1[:, :], (64, 64), name="w1")
            w2a = load(cst, W2[0:64, :], (64, 64), name="w2a")
            w2b = load(cst, W2[64:128, :], (64, 64), name="w2b")
            w3t = load(cst, W3[:, :], (64, 64), name="w3")
            w4t = load(cst, W4[:, :], (64, 64), name="w4")
            w5a = load(cst, W5[0:64, :], (64, 1), name="w5a")
            w5b = load(cst, W5[64:128, :], (64, 1), name="w5b")
            relt = load(cst, rel_emb[:, :], (64, 64), name="relt")
            b1col = load(cst, b1c[:, :], (64, 1), name="b1col")
            b2row = load(cst, b2r[:, :], (1, 64), name="b2row")
            gcol = load(cst, gammac[:, :], (64, 1), name="gcol")
            bcol = load(cst, betac[:, :], (64, 1), name="bcol")

            # ---- phase 1: weight folding (fp32, tiny) ----
            def transpose_to(pool, src_t, k, m, name):
                pst = ps.tile([m, k], F32, tag="ps")
                ident = id64 if k == 64 else id128
                mm(pst[:], src_t[:], ident[:, 0:k], is_transpose=True)
                dst = pool.tile([m, k], F32, tag=name)
                nc.scalar.copy(dst[:], pst[:])
                return dst

            w3T = transpose_to(cst, w3t, 64, 64, "w3T")
            w4T = transpose_to(cst, w4t, 64, 64, "w4T")
            w1T = transpose_to(cst, w1t, 64, 64, "w1T")
            relT = transpose_to(cst, relt, 64, 64, "relT")

            def mm_to_sbuf(pool, lhsT, rhs, m, n, name, dtype=F32):
                pst = ps.tile([m, n], F32, tag="ps")
                mm(pst[:], lhsT, rhs)
                dst = pool.tile([m, n], dtype, tag=name)
                nc.scalar.copy(dst[:], pst[:])
                return dst

            v3b = mm_to_sbuf(cst, w3T[:], w5a[:], 64, 1, "v3b", BF16)   # [64,1]
            v4 = mm_to_sbuf(cst, w4T[:], w5b[:], 64, 1, "v4")           # [64,1]
            w1s = mm_to_sbuf(cst, w1T[:], ones64[:], 64, 1, "w1s")      # [64,1]
            b1s = mm_to_sbuf(cst, b1col[:], ones64[:], 1, 1, "b1s")     # [1,1]

            # v4 broadcast tile [128, 64] fp32 (for the tail-table build)
            psv4r = ps.tile([1, 64], F32, tag="ps")
            mm(psv4r[:], v4[:], id64[:], is_transpose=True)
            v4row = cst.tile([1, 64], F32, tag="v4row")
            nc.scalar.copy(v4row[:], psv4r[:])
            v4tile = cst.tile([128, 64], F32, tag="v4tile")
            nc.gpsimd.partition_broadcast(v4tile[:], v4row[:])

            b1s_tile = cst.tile([128, 1], F32, tag="b1stile")
            nc.gpsimd.partition_broadcast(b1s_tile[:], b1s[:])

            b2tile = cst.tile([128, 64], F32, tag="b2tile")
            nc.gpsimd.partition_broadcast(b2tile[:], b2row[:])

            # bf16 weights for the edge/output matmuls
            w2ab = cst.tile([64, 64], BF16, tag="w2ab")
            nc.scalar.copy(w2ab[:], w2a[:])
            w2bb = cst.tile([64, 64], BF16, tag="w2bb")
            nc.scalar.copy(w2bb[:], w2b[:])

            # M_T = rel_emb.T * w1s (per-partition scale), bf16
            MTb = cst.tile([64, 64], BF16, tag="MTb")
            nc.vector.tensor_scalar(MTb[:], relT[:], w1s[:], None, OP.mult)

            # DVE fences: absorb preload DMA waits so hot-loop compact DVE
            # ops never need more than one embedded wait.
            for fi, ft in enumerate((iota128b, hrelC_t, v4tile, b2tile,
                                     gcol, bcol)):
                np_ = ft.shape[0]
                fj = cst.tile([np_, 1], F32, tag=f"fj{fi}")
                nc.vector.tensor_copy(fj[:], ft[0:np_, 0:1])
            fj16 = cst.tile([128, 1], I16, tag="fj16")
            nc.vector.tensor_copy(fj16[:], tails_t[:, 0:1])

            # ---- phase 1b: tail table build ----
            NIT = TROWS // (TCH * 128)
            for it in range(NIT):
                base = it * TCH * 128
                tch = tbp.tile([128, TCH, 64], F32, tag="tch")
                src = tail_pad[base:base + TCH * 128, :].rearrange(
                    "(c p) d -> p c d", p=128)
                nc.sync.dma_start(out=tch[:], in_=src)
                tbb = tbp.tile([128, TCH, 128], BF16, tag="tbb")
                nc.vector.memset(tbb[:, :, 65:128], 0.0)
                s4f = tbp.tile([128, TCH], F32, tag="s4f")
                for c in range(TCH):
                    junk = sb.tile([128, 64], F32, tag="junk")
                    nc.vector.scalar_tensor_tensor(
                        out=junk[:], in0=tch[:, c, :], scalar=0.0,
                        in1=v4tile[:], op0=OP.bypass, op1=OP.mult,
                        accum_out=s4f[:, c:c + 1])
                nc.vector.tensor_copy(tbb[:, :, 0:64], tch[:])
                nc.vector.tensor_copy(tbb[:, :, 64], s4f[:])
                dst = tailB[base:base + TCH * 128, :].rearrange(
                    "(c p) d -> p c d", p=128)
                nc.sync.dma_start(out=dst, in_=tbb[:])

            # ---- phase 2+3 fused: per-window pipeline ----
            ybuf = cst.tile([128, NWIN, 64], BF16, tag="ybuf")
            pStat = psS.tile([64, 2], F32, tag="pStat")
            drugTs = []

            NCH = (NBW + GCH - 1) // GCH
            for w in range(NWIN):
                # window drug prep -> SW = [Sb | s3] bf16 [128, 65]
                dchunk = sb.tile([128, 64], F32, tag="dchunk")
                nc.sync.dma_start(out=dchunk[:],
                                  in_=drug_rows[w * 128:(w + 1) * 128, :])
                psDT = ps.tile([64, 128], F32, tag="ps")
                mm(psDT[:], dchunk[:], id128[:], is_transpose=True)
                dTb = cst.tile([64, 128], BF16, tag=f"drugT{w}")
                nc.scalar.copy(dTb[:], psDT[:])
                drugTs.append(dTb)
                SW = swp.tile([128, 65], BF16, tag="SW")
                psSb = ps.tile([128, 64], F32, tag="ps")
                mm(psSb[:], dTb[:], MTb[:])
                nc.scalar.activation(SW[:, 0:64], psSb[:], AF.Identity,
                                     bias=b1s_tile[:], scale=1.0)
                psS3 = ps.tile([128, 1], F32, tag="ps")
                mm(psS3[:], dTb[:], v3b[:])
                nc.scalar.copy(SW[:, 64:65], psS3[:])

                # gather this window's tail rows [128, NBW, 128] bf16
                tg = tgp.tile([128, NBW, 128], BF16, tag="tg")
                gat_i = None
                for g0 in range(0, NBW, GCH):
                    gn = min(GCH, NBW - g0) * 128
                    io = (w * NBW + g0) * 8
                    gat_i = nc.gpsimd.dma_gather(
                        out_ap=tg[:, g0:g0 + gn // 128, :],
                        in_ap=tailB[:, :],
                        idxs_ap=(tails_t[:, io:io + gn // 16] if SIMIDX
                                 else tails_t[0:16, io:io + gn // 16]),
                        num_idxs=gn,
                        num_idxs_reg=gn,
                        elem_size=128,
                        queue_num=(w * NCH + g0 // GCH) % NQ,
                    )

                # stream this window's one-hots
                ohTw = ohtp.tile([128, NBW, 128], OHT_DT, tag="ohTw")
                nc.sync.dma_start(out=ohTw[:],
                                  in_=ohT_in[:, w * NBW:(w + 1) * NBW, :])
                relw = relp.tile([128, NBW, 64], REL_DT, tag="relw")
                nc.sync.dma_start(out=relw[:],
                                  in_=relOH_in[:, w * NBW:(w + 1) * NBW, :])
                ohEw = ohep.tile([128, NBW, 128], OHT_DT, tag="ohEw")
                nc.sync.dma_start(out=ohEw[:],
                                  in_=ohE_in[:, w * NBW:(w + 1) * NBW, :])

                pU = psU.tile([128, 65], F32, tag="pU")

                for j0 in range(0, NBW, GRP):
                    g = min(GRP, NBW - j0)
                    sv16 = grp.tile([128, GRP], F32, tag="sv16")
                    att16 = grp.tile([128, GRP], F32, tag="att16")
                    s4c16 = grp.tile([128, GRP], F32, tag="s4c16")
                    # s4 junction copy (absorbs the gather DMA wait)
                    s4i = nc.scalar.copy(s4c16[:, 0:g], tg[:, j0:j0 + g, 64])
                    dep(s4i, gat_i)
                    for q in range(0, g, GEXT):
                        psG4 = psG.tile([128, GEXT, 65], F32, tag="psG4")
                        for i in range(GEXT):
                            j = j0 + q + i
                            mm(psG4[:, i, :], ohTw[:, j, :], SW[:],
                               start=True, stop=True)
                        sel4 = selp.tile([128, GEXT, 64], BF16, tag="sel4")
                        nc.vector.tensor_tensor(
                            out=sel4[:], in0=psG4[:, :, 0:64],
                            in1=relw[:, j0 + q:j0 + q + GEXT, :], op=OP.mult)
                        nc.vector.tensor_reduce(
                            out=sv16[:, q:q + GEXT], in_=sel4[:],
                            axis=mybir.AxisListType.X, op=OP.add)
                        nc.vector.tensor_tensor(
                            out=att16[:, q:q + GEXT], in0=psG4[:, :, 64],
                            in1=s4c16[:, q:q + GEXT], op=OP.add)
                    # group scalar pipeline [128, g]
                    l16 = grp.tile([128, GRP], F32, tag="l16")
                    nc.vector.scalar_tensor_tensor(
                        out=l16[:, 0:g], in0=att16[:, 0:g], scalar=SLOPE,
                        in1=att16[:, 0:g], op0=OP.mult, op1=OP.max)
                    p16 = grp.tile([128, GRP], F32, tag="p16")
                    nc.scalar.activation(p16[:, 0:g], l16[:, 0:g], AF.Exp)
                    svc16 = grp.tile([128, GRP], F32, tag="svc16")
                    nc.vector.tensor_scalar(svc16[:, 0:g], sv16[:, 0:g], 1e-12,
                                            None, OP.add)
                    wg16 = grp.tile([128, GRP], F32, tag="wg16")
                    nc.vector.tensor_tensor(out=wg16[:, 0:g], in0=p16[:, 0:g],
                                            in1=svc16[:, 0:g], op=OP.mult)
                    rc16 = grp.tile([128, GRP], F32, tag="rc16")
                    nc.vector.reciprocal(rc16[:, 0:g], svc16[:, 0:g])
                    nc.scalar.copy(tg[:, j0:j0 + g, 64], rc16[:, 0:g])
                    for jj in range(g):
                        j = j0 + jj
                        wt = wtp.tile([128, 65], BF16, tag="wt")
                        nc.scalar.activation(wt[:], tg[:, j, 0:65], AF.Identity,
                                             scale=wg16[:, jj:jj + 1])
                        mm(pU[:], ohEw[:, j, :], wt[:],
                           start=(j == 0), stop=(j == NBW - 1))

                # window reduction -> neigh -> y -> stats
                dsafe = sb.tile([128, 1], F32, tag="dsafe")
                nc.vector.tensor_scalar(dsafe[:], pU[:, 64:65], 1e-30, None,
                                        OP.add)
                recip = sb.tile([128, 1], F32, tag="recip")
                nc.vector.reciprocal(recip[:], dsafe[:])
                nw = sb.tile([128, 64], F32, tag="nw")
                nc.vector.tensor_scalar(nw[:], pU[:, 0:64], recip[:], None,
                                        OP.mult)
                psNT = ps.tile([64, 128], F32, tag="psT")
                mm(psNT[:], nw[:], id128[:], is_transpose=True)
                nT = sb.tile([64, 128], BF16, tag="nT")
                nc.scalar.copy(nT[:], psNT[:])
                pY = ps.tile([128, 64], F32, tag="ps")
                mm(pY[:], drugTs[w][:], w2ab[:], start=True, stop=False)
                mm(pY[:], nT[:], w2bb[:], start=False, stop=True)
                nc.vector.tensor_tensor(out=ybuf[:, w, :], in0=pY[:],
                                        in1=b2tile[:], op=OP.add)
                sq = sb.tile([128, 64], BF16, tag="sq")
                nc.vector.tensor_tensor(out=sq[:], in0=ybuf[:, w, :],
                                        in1=ybuf[:, w, :], op=OP.mult)
                mm(pStat[:, 0:1], ybuf[:, w, :], ones128b[:],
                   start=(w == 0), stop=(w == NWIN - 1))
                mm(pStat[:, 1:2], sq[:], ones128b[:],
                   start=(w == 0), stop=(w == NWIN - 1))

            # ---- phase 4: batchnorm ----
            statsb = sb.tile([64, 2], F32, tag="statsb")
            nc.scalar.copy(statsb[:], pStat[:])
            nc.sync.dma_start(out=cc_in[:, :], in_=statsb[:])
            nc.gpsimd.collective_compute(
                "AllReduce", OP.add, replica_groups=[list(range(NC))],
                ins=[cc_in[:, :]], outs=[cc_out[:, :]])
            statsg = sb.tile([64, 2], F32, tag="statsg")
            nc.sync.dma_start(out=statsg[:], in_=cc_out[:, :])
            fjs = sb.tile([64, 1], F32, tag="fjs")
            nc.vector.tensor_copy(fjs[:], statsg[:, 0:1])

            mean = sb.tile([64, 1], F32, tag="mean")
            nc.vector.tensor_scalar(mean[:], statsg[:, 0:1], 1.0 / N_DRUG,
                                    None, OP.mult)
            ex2 = sb.tile([64, 1], F32, tag="ex2")
            nc.vector.tensor_scalar(ex2[:], statsg[:, 1:2], 1.0 / N_DRUG,
                                    None, OP.mult)
            msq = sb.tile([64, 1], F32, tag="msq")
            nc.vector.tensor_tensor(out=msq[:], in0=mean[:], in1=mean[:],
                                    op=OP.mult)
            var = sb.tile([64, 1], F32, tag="var")
            nc.vector.tensor_tensor(out=var[:], in0=ex2[:], in1=msq[:],
                                    op=OP.subtract)
            vare = sb.tile([64, 1], F32, tag="vare")
            nc.vector.tensor_scalar(vare[:], var[:], EPS, None, OP.add)
            sd = sb.tile([64, 1], F32, tag="sd")
            nc.scalar.activation(sd[:], vare[:], AF.Sqrt)
            rstd = sb.tile([64, 1], F32, tag="rstd")
            nc.vector.reciprocal(rstd[:], sd[:])
            scalec = sb.tile([64, 1], F32, tag="scalec")
            nc.vector.tensor_tensor(out=scalec[:], in0=gcol[:], in1=rstd[:],
                                    op=OP.mult)
            tmp = sb.tile([64, 1], F32, tag="tmp")
            nc.vector.tensor_tensor(out=tmp[:], in0=mean[:], in1=scalec[:],
                                    op=OP.mult)
            shiftc = sb.tile([64, 1], F32, tag="shiftc")
            nc.vector.tensor_tensor(out=shiftc[:], in0=bcol[:], in1=tmp[:],
                                    op=OP.subtract)

            def col_to_tile(col, name, dtype=F32):
                pst = ps.tile([1, 64], F32, tag="ps")
                mm(pst[:], col[:], id64[:], is_transpose=True)
                row = sb.tile([1, 64], dtype, tag=name + "r")
                nc.scalar.copy(row[:], pst[:])
                t = cst.tile([128, 64], dtype, tag=name)
                nc.gpsimd.partition_broadcast(t[:], row[:])
                return t

            scale_t = col_to_tile(scalec, "scalet", BF16)
            shift_t = col_to_tile(shiftc, "shiftt", F32)
            for fi, ft in enumerate((scale_t, shift_t)):
                fjt = sb.tile([128, 1], F32, tag=f"fjt{fi}")
                nc.vector.tensor_copy(fjt[:], ft[:, 0:1])

            for w in range(NWIN):
                o1 = sb.tile([128, 64], F32, tag="o1")
                nc.vector.tensor_tensor(out=o1[:], in0=ybuf[:, w, :],
                                        in1=scale_t[:], op=OP.mult)
                o2 = sb.tile([128, 64], F32, tag="o2")
                nc.vector.tensor_tensor(out=o2[:], in0=o1[:], in1=shift_t[:],
                                        op=OP.add)
                nc.sync.dma_start(out=out_rows[w * 128:(w + 1) * 128, :],
                                  in_=o2[:])

        for _rep in range(int(os.environ.get('BASS_REPEAT', '1'))):
            body()

        for p in (psS, psU, psG, ps, wtp, ohep, selp, grp, swp, relp, ohtp,
                  tgp, tbp, sb, cst):
            p.release()

    nc.finalize()
    return nc


def _host_prep(DKG):
    """Sort edges by head, shard by head range, build per-core slot arrays
    (index-only preprocessing)."""
    heads = np.asarray(DKG[:, 0], dtype=np.int64)
    tails = np.asarray(DKG[:, 1], dtype=np.int64)
    rels = np.asarray(DKG[:, 2], dtype=np.int64)

    order = np.argsort(heads, kind="stable")
    hs, ts, rs = heads[order], tails[order], rels[order]

    core_lo = np.searchsorted(hs, HPC * np.arange(NC), side="left")
    core_hi = np.searchsorted(hs, HPC * (np.arange(NC) + 1), side="left")

    winb = np.searchsorted(hs, WIN * np.arange(NC * NWIN), side="left")
    wine = np.searchsorted(hs, WIN * (np.arange(NC * NWIN) + 1), side="left")
    maxw = int((wine - winb).max())
    NBW = max(1, (maxw + 127) // 128)
    NBW = (NBW + GEXT - 1) // GEXT * GEXT   # multiple of the extraction chunk
    NB = NWIN * NBW
    S = NB * 128

    oht_dt = np.dtype(mybir.dt.np(OHT_DT))
    rel_dt = np.dtype(mybir.dt.np(REL_DT))
    per_core = []
    for c in range(NC):
        lo, hi = core_lo[c], core_hi[c]
        ch, ct, cr = hs[lo:hi], ts[lo:hi], rs[lo:hi]
        hrel = np.full(S, PAD_H, np.float32)
        rel = np.full(S, -1, np.int64)
        tail = np.zeros(S, np.int64)
        base = c * HPC
        for w in range(NWIN):
            wl = np.searchsorted(ch, base + w * WIN, side="left")
            wh = np.searchsorted(ch, base + (w + 1) * WIN, side="left")
            n = wh - wl
            o = w * NBW * 128
            hrel[o:o + n] = (ch[wl:wh] - base - w * WIN).astype(np.float32)
            rel[o:o + n] = cr[wl:wh]
            tail[o:o + n] = ct[wl:wh]
        hrelC = hrel.reshape(NB, 128).T.copy()
        t16 = tail.reshape(S // 16, 16).T.astype(np.int16)          # [16, S/16]
        t16r = np.tile(t16, (8, 1)).copy()                          # [128, S/16]
        # ohT [head k, blk b, edge m] = (hrel[b*128+m] == k), fp8
        hrel_bm = hrel.reshape(NB, 128)                             # [b, m]
        ohT = (np.arange(128, dtype=np.float32)[:, None, None]
               == hrel_bm[None, :, :]).astype(oht_dt)
        # relOH [edge p, blk b, r] = (rel[b*128+p] == r), fp8
        rel_bp = rel.reshape(NB, 128).T                             # [p, b]
        relOH = (rel_bp[:, :, None]
                 == np.arange(64, dtype=np.int64)[None, None, :]).astype(rel_dt)
        # ohE [edge p, blk b, head k] = (hrel[b*128+p] == k), fp8
        ohE = (hrelC[:, :, None]
               == np.arange(128, dtype=np.float32)[None, None, :]).astype(oht_dt)
        per_core.append(dict(hrelC=hrelC, tails16=t16r, ohT=ohT, relOH=relOH,
                             ohE=ohE))
    return NBW, per_core


def prepare(X, DKG, drug_emb, rel_emb, tail_emb, W1, b1, W2, b2, gamma, beta,
            W3, W4, W5):
    f = np.float32
    bf = np.dtype(mybir.dt.np(BF16))
    NBW, per_core = _host_prep(np.asarray(DKG))
    nc = _build_nc(NBW)

    consts = dict(
        c_iota128b=np.broadcast_to(np.arange(128, dtype=f),
                                   (128, 128)).astype(bf),
        c_id128=np.eye(128, dtype=f),
        c_id64=np.eye(64, dtype=f),
        c_id128b=np.eye(128, dtype=f).astype(bf),
        c_ones64=np.ones((64, 1), f),
        c_ones128b=np.ones((128, 1), f).astype(bf),
    )
    tp = np.zeros((TROWS, D), f)
    tp[:N_TAIL] = np.asarray(tail_emb, f)
    weights = dict(
        tail_pad=tp,
        rel_emb=np.asarray(rel_emb, f),
        W1=np.asarray(W1, f), W2=np.asarray(W2, f), W3=np.asarray(W3, f),
        W4=np.asarray(W4, f), W5=np.asarray(W5, f),
        b1c=np.asarray(b1, f).reshape(D, 1),
        b2r=np.asarray(b2, f).reshape(1, D),
        gammac=np.asarray(gamma, f).reshape(D, 1),
        betac=np.asarray(beta, f).reshape(D, 1),
    )
    de = np.asarray(drug_emb, f)
    in_maps = []
    for c in range(NC):
        dr = np.zeros((DROWS, D), f)
        dr[:HPC] = de[c * HPC:(c + 1) * HPC]
        m = dict(weights)
        m.update(consts)
        m["drug_rows"] = dr
        pc = per_core[c]
        m["hrelC"] = pc["hrelC"]
        m["tails16"] = pc["tails16"]
        m["ohT"] = pc["ohT"]
        m["relOH"] = pc["relOH"]
        m["ohE"] = pc["ohE"]
        in_maps.append(m)
    return nc, in_maps


def kernel(X, DKG, drug_emb, rel_emb, tail_emb, W1, b1, W2, b2, gamma, beta,
           W3, W4, W5):
    X = np.asarray(X)
    nc, in_maps = prepare(X, DKG, drug_emb, rel_emb, tail_emb, W1, b1, W2, b2,
                          gamma, beta, W3, W4, W5)

    res = run_bass_kernel_spmd(nc, in_maps, core_ids=list(range(NC)))
    global LAST_RESULT
    LAST_RESULT = res
    out = np.concatenate([np.asarray(res.results[c]["out_rows"][:HPC], np.float32)
                          for c in range(NC)], axis=0)
    return out, X


LAST_RESULT = None


# revision 17
# speedup vs baseline: 2.3578x; 1.0038x over previous
"""GAT layer (gnn_message_passing) Trainium2 kernel — v2.

Math (after algebraic simplification of the reference):
  v3 = W3 @ W5[:64];  v4 = W4 @ W5[64:]           # [64]
  s3 = drug_emb @ v3                               # [N_DRUG]
  s4 = tail_emb @ v4                               # [N_TAIL]
  Sb = drug_emb @ (rel_emb * (W1 @ 1)).T + sum(b1) # [N_DRUG, N_REL]
  att_e  = leaky_relu(s3[h_e] + s4[t_e])
  p_e    = exp(att_e)            (softmax max-shift dropped: shift-invariant)
  w_e    = p_e * Sb[h_e, r_e]
  U[h]   = sum_e w_e * tail_emb[t_e];  den[h] = sum_e p_e
  neigh  = U / den
  y      = [drug_emb | neigh] @ W2 + b2;  out = batchnorm(y) (training stats)

Sharding: edges sorted by head on the host (index-only preprocessing);
8 cores own disjoint 2500-head ranges, so segment stats complete locally.
Only the 64x2 batchnorm statistics are all-reduced.

v2 device strategy (per core, per 128-head window, 128-edge blocks):
  - tailB DRAM table [20480, 128] bf16 rows [tail|s4|pad], built on device
    once; per-window dma_gather pulls 256B rows (s4 rides along).
  - host ships one-hot matrices as fp8: ohT (head one-hot, lhsT for the
    per-edge SW-row gather matmul) and relOH (rel one-hot for Sb column
    selection).
  - per block: 1 pG matmul (gather [Sb_row|s3] per edge), 1 fused
    tensor_scalar builds the wg-scaled scatter one-hot, 1 pU matmul
    accumulates [U|den] in PSUM.  den uses rhs col64 = 1/sv so that
    wg*(1/sv) = p.
  - per 4 blocks: one TT-mult + tensor_reduce extracts sv; one TT-add
    forms att.  Per 16 blocks: lrelu (on DVE), exp (ACT), reciprocal etc.
  - all edge-pass matmuls bf16/fp8 (single HW pass vs fp32's two).
"""

import os

import numpy as np

import concourse.bacc as bacc
import concourse.bass as bass
import concourse.tile as tile
from concourse import mybir
from concourse.bass_utils import run_bass_kernel_spmd

F32 = mybir.dt.float32
BF16 = mybir.dt.bfloat16
F8 = mybir.dt.float8e4
I16 = mybir.dt.int16
AF = mybir.ActivationFunctionType
OP = mybir.AluOpType

N_DRUG = 20000
N_TAIL = 20000
N_REL = 64
D = 64
NC = 8
HPC = N_DRUG // NC          # heads per core
WIN = 128                   # heads per window
NWIN = (HPC + WIN - 1) // WIN  # windows per core (20)
DROWS = NWIN * WIN          # padded drug rows per core (2560)
TROWS = 20480               # padded tail rows (160 chunks of 128)
TCH = 16                    # tail-table chunks per iteration
EPS = 1e-5
SLOPE = 0.01
GEXT = 4                    # blocks per extraction chunk (PSUM-bank bound)
GRP = 16                    # blocks per batched-scalar group
NQ = int(os.environ.get("K_NQ", "1"))      # SWDGE queues for dma_gather
GCH = int(os.environ.get("K_GCH", "8"))    # blocks per dma_gather call
SIMIDX = os.environ.get("K_SIMIDX", "0") == "1"  # full-128 idx AP (CoreSim)
DMA_SCRATCH = int(os.environ.get("K_SCRATCH", "16384"))
PAD_H = 999.0               # hrel sentinel for padded slots (no one-hot match)
F8OHT = os.environ.get("K_F8OHT", "0") == "1"   # ship ohT as fp8 (else bf16)
F8REL = os.environ.get("K_F8REL", "0") == "1"   # ship relOH as fp8 (else bf16)
OHT_DT = F8 if F8OHT else BF16
REL_DT = F8 if F8REL else BF16


def _build_nc(NBW: int):
    """Build the Bass module. NBW = 128-edge blocks per 128-head window."""
    NB = NWIN * NBW          # blocks per core
    S = NB * 128             # edge slots per core

    nc = bacc.Bacc(None, num_devices=NC, num_swdge_queues=NQ,
                   dynamic_dma_scratch_size=DMA_SCRATCH)

    # ---- I/O ----
    def inp(name, shape, dtype=F32):
        return nc.declare_dram_parameter(name, list(shape), dtype, isOutput=False)

    tail_pad = inp("tail_pad", (TROWS, D))
    drug_rows = inp("drug_rows", (DROWS, D))
    rel_emb = inp("rel_emb", (N_REL, D))
    W1 = inp("W1", (D, D))
    W2 = inp("W2", (2 * D, D))
    W3 = inp("W3", (D, D))
    W4 = inp("W4", (D, D))
    W5 = inp("W5", (2 * D, 1))
    b1c = inp("b1c", (D, 1))
    b2r = inp("b2r", (1, D))
    gammac = inp("gammac", (D, 1))
    betac = inp("betac", (D, 1))

    hrelC = inp("hrelC", (128, NB))          # hrel per slot (pads = PAD_H)
    tails16 = inp("tails16", (128, S // 16), I16)
    # packed per-block one-hots: [ohT(128) | ohE(128) | relOH(64)]
    #   ohT  [head k, blk, edge m] — lhsT for the per-edge SW-row gather
    #   ohE  [edge p, blk, head k] — lhsT for the pU scatter
    #   relOH[edge p, blk, rel r]  — Sb column selection
    ohall_in = inp("ohall", (128, NB, 320), OHT_DT)

    c_iota128b = inp("c_iota128b", (128, 128), BF16)
    c_id128 = inp("c_id128", (128, 128))
    c_id64 = inp("c_id64", (64, 64))
    c_id128b = inp("c_id128b", (128, 128), BF16)
    c_ones64 = inp("c_ones64", (64, 1))
    c_ones128b = inp("c_ones128b", (128, 1), BF16)

    out_rows = nc.declare_dram_parameter("out_rows", [DROWS, D], F32, isOutput=True)

    # DRAM scratch: bf16 tail table rows [tail(64) | s4 | junk(63)]
    tailB = nc.dram_tensor("tailB", [TROWS, 128], BF16)

    # collective bounce buffers
    cc_in = nc.dram_tensor("cc_in", [D, 2], F32)
    cc_out = nc.dram_tensor("cc_out", [D, 2], F32, addr_space="Shared")

    with tile.TileContext(nc) as tc:
        cst = tc.alloc_tile_pool(name="cst", bufs=1)
        sb = tc.alloc_tile_pool(name="sb", bufs=2)
        tbp = tc.alloc_tile_pool(name="tbp", bufs=2)
        tgp = tc.alloc_tile_pool(name="tgp", bufs=2)
        ohtp = tc.alloc_tile_pool(name="ohtp", bufs=2)
        relp = tc.alloc_tile_pool(name="relp", bufs=2)
        swp = tc.alloc_tile_pool(name="swp", bufs=2)
        grp = tc.alloc_tile_pool(name="grp", bufs=2)
        selp = tc.alloc_tile_pool(name="selp", bufs=2)
        wtp = tc.alloc_tile_pool(name="wtp", bufs=6)
        ps = tc.alloc_tile_pool(name="ps", bufs=1, space="PSUM")
        psG = tc.alloc_tile_pool(name="psG", bufs=3, space="PSUM")
        psU = tc.alloc_tile_pool(name="psU", bufs=2, space="PSUM")
        psS = tc.alloc_tile_pool(name="psS", bufs=1, space="PSUM")

        def body():
            # absorber: first DVE instruction after the init barrier must
            # carry no data wait.
            dve0 = cst.tile([128, 1], F32, tag="dve0")
            nc.vector.memset(dve0[:], 0.0)

            def mm(out, lhsT, rhs, start=True, stop=True, is_transpose=None):
                return nc.tensor.matmul(out, lhsT, rhs, start=start, stop=stop,
                                        is_transpose=is_transpose,
                                        skip_group_check=True)

            import bass_rust as _br

            def dep(a, b):
                _br.add_dep_helper(a.ins, b.ins, sync=True, reason="wait-routing")

            def load(pool, src_ap, shape, dtype=F32, name=None):
                t = pool.tile(list(shape), dtype, tag=name)
                nc.sync.dma_start(out=t[:], in_=src_ap)
                return t

            # ---- constants into SBUF ----
            iota128b = load(cst, c_iota128b[:, :], (128, 128), BF16, "iota128b")
            id128 = load(cst, c_id128[:, :], (128, 128), name="id128")
            id64 = load(cst, c_id64[:, :], (64, 64), name="id64")
            id128b = load(cst, c_id128b[:, :], (128, 128), BF16, "id128b")
            ones64 = load(cst, c_ones64[:, :], (64, 1), name="ones64")
            ones128b = load(cst, c_ones128b[:, :], (128, 1), BF16, "ones128b")

            hrelC_t = load(cst, hrelC[:, :], (128, NB), name="hrelC")
            tails_t = load(cst, tails16[:, :], (128, S // 16), I16, name="tails")

            w1t = load(cst, W1[:, :], (64, 64), name="w1")
            w2a = load(cst, W2[0:64, :], (64, 64), name="w2a")
            w2b = load(cst, W2[64:128, :], (64, 64), name="w2b")
            w3t = load(cst, W3[:, :], (64, 64), name="w3")
            w4t = load(cst, W4[:, :], (64, 64), name="w4")
            w5a = load(cst, W5[0:64, :], (64, 1), name="w5a")
            w5b = load(cst, W5[64:128, :], (64, 1), name="w5b")
            relt = load(cst, rel_emb[:, :], (64, 64), name="relt")
            b1col = load(cst, b1c[:, :], (64, 1), name="b1col")
            b2row = load(cst, b2r[:, :], (1, 64), name="b2row")
            gcol = load(cst, gammac[:, :], (64, 1), name="gcol")
            bcol = load(cst, betac[:, :], (64, 1), name="bcol")

            # ---- phase 1: weight folding (fp32, tiny) ----
            def transpose_to(pool, src_t, k, m, name):
                pst = ps.tile([m, k], F32, tag="ps")
                ident = id64 if k == 64 else id128
                mm(pst[:], src_t[:], ident[:, 0:k], is_transpose=True)
                dst = pool.tile([m, k], F32, tag=name)
                nc.scalar.copy(dst[:], pst[:])
                return dst

            w3T = transpose_to(cst, w3t, 64, 64, "w3T")
            w4T = transpose_to(cst, w4t, 64, 64, "w4T")
            w1T = transpose_to(cst, w1t, 64, 64, "w1T")
            relT = transpose_to(cst, relt, 64, 64, "relT")

            def mm_to_sbuf(pool, lhsT, rhs, m, n, name, dtype=F32):
                pst = ps.tile([m, n], F32, tag="ps")
                mm(pst[:], lhsT, rhs)
                dst = pool.tile([m, n], dtype, tag=name)
                nc.scalar.copy(dst[:], pst[:])
                return dst

            v3b = mm_to_sbuf(cst, w3T[:], w5a[:], 64, 1, "v3b", BF16)   # [64,1]
            v4 = mm_to_sbuf(cst, w4T[:], w5b[:], 64, 1, "v4")           # [64,1]
            w1s = mm_to_sbuf(cst, w1T[:], ones64[:], 64, 1, "w1s")      # [64,1]
            b1s = mm_to_sbuf(cst, b1col[:], ones64[:], 1, 1, "b1s")     # [1,1]
            # +1e-12 biases Sb so the per-edge reciprocal below never hits 0
            b1sp = cst.tile([1, 1], F32, tag="b1sp")
            nc.vector.tensor_scalar(b1sp[:], b1s[:], 1e-12, None, OP.add)

            # v4 broadcast tile [128, 64] fp32 (for the tail-table build)
            psv4r = ps.tile([1, 64], F32, tag="ps")
            mm(psv4r[:], v4[:], id64[:], is_transpose=True)
            v4row = cst.tile([1, 64], F32, tag="v4row")
            nc.scalar.copy(v4row[:], psv4r[:])
            v4tile = cst.tile([128, 64], F32, tag="v4tile")
            nc.gpsimd.partition_broadcast(v4tile[:], v4row[:])

            b1s_tile = cst.tile([128, 1], F32, tag="b1stile")
            nc.gpsimd.partition_broadcast(b1s_tile[:], b1sp[:])

            b2tile = cst.tile([128, 64], F32, tag="b2tile")
            nc.gpsimd.partition_broadcast(b2tile[:], b2row[:])

            # bf16 weights for the edge/output matmuls
            w2ab = cst.tile([64, 64], BF16, tag="w2ab")
            nc.scalar.copy(w2ab[:], w2a[:])
            w2bb = cst.tile([64, 64], BF16, tag="w2bb")
            nc.scalar.copy(w2bb[:], w2b[:])

            # M_T = rel_emb.T * w1s (per-partition scale), bf16
            MTb = cst.tile([64, 64], BF16, tag="MTb")
            nc.vector.tensor_scalar(MTb[:], relT[:], w1s[:], None, OP.mult)

            # DVE fences: absorb preload DMA waits so hot-loop compact DVE
            # ops never need more than one embedded wait.
            for fi, ft in enumerate((iota128b, hrelC_t, v4tile, b2tile,
                                     gcol, bcol)):
                np_ = ft.shape[0]
                fj = cst.tile([np_, 1], F32, tag=f"fj{fi}")
                nc.vector.tensor_copy(fj[:], ft[0:np_, 0:1])
            fj16 = cst.tile([128, 1], I16, tag="fj16")
            nc.vector.tensor_copy(fj16[:], tails_t[:, 0:1])

            # ---- phase 1b: tail table build ----
            NIT = TROWS // (TCH * 128)
            for it in range(NIT):
                base = it * TCH * 128
                tch = tbp.tile([128, TCH, 64], F32, tag="tch")
                src = tail_pad[base:base + TCH * 128, :].rearrange(
                    "(c p) d -> p c d", p=128)
                nc.sync.dma_start(out=tch[:], in_=src)
                tbb = tbp.tile([128, TCH, 128], BF16, tag="tbb")
                nc.vector.memset(tbb[:, :, 65:128], 0.0)
                s4f = tbp.tile([128, TCH], F32, tag="s4f")
                for c in range(TCH):
                    junk = sb.tile([128, 64], F32, tag="junk")
                    nc.vector.scalar_tensor_tensor(
                        out=junk[:], in0=tch[:, c, :], scalar=0.0,
                        in1=v4tile[:], op0=OP.bypass, op1=OP.mult,
                        accum_out=s4f[:, c:c + 1])
                nc.vector.tensor_copy(tbb[:, :, 0:64], tch[:])
                nc.vector.tensor_copy(tbb[:, :, 64], s4f[:])
                dst = tailB[base:base + TCH * 128, :].rearrange(
                    "(c p) d -> p c d", p=128)
                nc.sync.dma_start(out=dst, in_=tbb[:])

            # ---- phase 2+3 fused: per-window pipeline ----
            ybuf = cst.tile([128, NWIN, 64], BF16, tag="ybuf")
            pStat = psS.tile([64, 2], F32, tag="pStat")
            drugTs = []

            NCH = (NBW + GCH - 1) // GCH
            for w in range(NWIN):
                # window drug prep -> SW = [Sb | s3] bf16 [128, 65]
                dchunk = sb.tile([128, 64], F32, tag="dchunk")
                nc.sync.dma_start(out=dchunk[:],
                                  in_=drug_rows[w * 128:(w + 1) * 128, :])
                psDT = ps.tile([64, 128], F32, tag="ps")
                mm(psDT[:], dchunk[:], id128[:], is_transpose=True)
                dTb = cst.tile([64, 128], BF16, tag=f"drugT{w}")
                nc.scalar.copy(dTb[:], psDT[:])
                drugTs.append(dTb)
                SW = swp.tile([128, 65], BF16, tag="SW")
                psSb = ps.tile([128, 64], F32, tag="ps")
                mm(psSb[:], dTb[:], MTb[:])
                nc.scalar.activation(SW[:, 0:64], psSb[:], AF.Identity,
                                     bias=b1s_tile[:], scale=1.0)
                psS3 = ps.tile([128, 1], F32, tag="ps")
                mm(psS3[:], dTb[:], v3b[:])
                nc.scalar.copy(SW[:, 64:65], psS3[:])

                # gather this window's tail rows [128, NBW, 128] bf16
                tg = tgp.tile([128, NBW, 128], BF16, tag="tg")
                gat_i = None
                for g0 in range(0, NBW, GCH):
                    gn = min(GCH, NBW - g0) * 128
                    io = (w * NBW + g0) * 8
                    gat_i = nc.gpsimd.dma_gather(
                        out_ap=tg[:, g0:g0 + gn // 128, :],
                        in_ap=tailB[:, :],
                        idxs_ap=(tails_t[:, io:io + gn // 16] if SIMIDX
                                 else tails_t[0:16, io:io + gn // 16]),
                        num_idxs=gn,
                        num_idxs_reg=gn,
                        elem_size=128,
                        queue_num=(w * NCH + g0 // GCH) % NQ,
                    )

                # stream this window's packed one-hots
                ohw = ohtp.tile([128, NBW, 320], OHT_DT, tag="ohw")
                nc.sync.dma_start(out=ohw[:],
                                  in_=ohall_in[:, w * NBW:(w + 1) * NBW, :])

                pU = psU.tile([128, 65], F32, tag="pU")

                for j0 in range(0, NBW, GRP):
                    g = min(GRP, NBW - j0)
                    sv16 = grp.tile([128, GRP], F32, tag="sv16")
                    att16 = grp.tile([128, GRP], F32, tag="att16")
                    s4c16 = grp.tile([128, GRP], F32, tag="s4c16")
                    # s4 junction copy (absorbs the gather DMA wait)
                    s4i = nc.scalar.copy(s4c16[:, 0:g], tg[:, j0:j0 + g, 64])
                    dep(s4i, gat_i)
                    for q in range(0, g, GEXT):
                        psG4 = psG.tile([128, GEXT, 65], F32, tag="psG4")
                        for i in range(GEXT):
                            j = j0 + q + i
                            mm(psG4[:, i, :], ohw[:, j, 0:128], SW[:],
                               start=True, stop=True)
                        sel4 = selp.tile([128, GEXT, 64], BF16, tag="sel4")
                        nc.vector.tensor_tensor(
                            out=sel4[:], in0=psG4[:, :, 0:64],
                            in1=ohw[:, j0 + q:j0 + q + GEXT, 256:320], op=OP.mult)
                        nc.vector.tensor_reduce(
                            out=sv16[:, q:q + GEXT], in_=sel4[:],
                            axis=mybir.AxisListType.X, op=OP.add)
                        nc.vector.tensor_tensor(
                            out=att16[:, q:q + GEXT], in0=psG4[:, :, 64],
                            in1=s4c16[:, q:q + GEXT], op=OP.add)
                    # group scalar pipeline [128, g]
                    l16 = grp.tile([128, GRP], F32, tag="l16")
                    nc.vector.scalar_tensor_tensor(
                        out=l16[:, 0:g], in0=att16[:, 0:g], scalar=SLOPE,
                        in1=att16[:, 0:g], op0=OP.mult, op1=OP.max)
                    p16 = grp.tile([128, GRP], F32, tag="p16")
                    nc.scalar.activation(p16[:, 0:g], l16[:, 0:g], AF.Exp)
                    wg16 = grp.tile([128, GRP], F32, tag="wg16")
                    nc.vector.tensor_tensor(out=wg16[:, 0:g], in0=p16[:, 0:g],
                                            in1=sv16[:, 0:g], op=OP.mult)
                    rc16 = grp.tile([128, GRP], F32, tag="rc16")
                    nc.vector.reciprocal(rc16[:, 0:g], sv16[:, 0:g])
                    nc.scalar.copy(tg[:, j0:j0 + g, 64], rc16[:, 0:g])
                    for jj in range(g):
                        j = j0 + jj
                        wt = wtp.tile([128, 65], BF16, tag="wt")
                        nc.scalar.activation(wt[:], tg[:, j, 0:65], AF.Identity,
                                             scale=wg16[:, jj:jj + 1])
                        mm(pU[:], ohw[:, j, 128:256], wt[:],
                           start=(j == 0), stop=(j == NBW - 1))

                # window reduction -> neigh -> y -> stats
                dsafe = sb.tile([128, 1], F32, tag="dsafe")
                nc.vector.tensor_scalar(dsafe[:], pU[:, 64:65], 1e-30, None,
                                        OP.add)
                recip = sb.tile([128, 1], F32, tag="recip")
                nc.vector.reciprocal(recip[:], dsafe[:])
                nw = sb.tile([128, 64], F32, tag="nw")
                nc.vector.tensor_scalar(nw[:], pU[:, 0:64], recip[:], None,
                                        OP.mult)
                psNT = ps.tile([64, 128], F32, tag="psT")
                mm(psNT[:], nw[:], id128[:], is_transpose=True)
                nT = sb.tile([64, 128], BF16, tag="nT")
                nc.scalar.copy(nT[:], psNT[:])
                pY = ps.tile([128, 64], F32, tag="ps")
                mm(pY[:], drugTs[w][:], w2ab[:], start=True, stop=False)
                mm(pY[:], nT[:], w2bb[:], start=False, stop=True)
                nc.vector.tensor_tensor(out=ybuf[:, w, :], in0=pY[:],
                                        in1=b2tile[:], op=OP.add)
                sq = sb.tile([128, 64], BF16, tag="sq")
                nc.vector.tensor_tensor(out=sq[:], in0=ybuf[:, w, :],
                                        in1=ybuf[:, w, :], op=OP.mult)
                mm(pStat[:, 0:1], ybuf[:, w, :], ones128b[:],
                   start=(w == 0), stop=(w == NWIN - 1))
                mm(pStat[:, 1:2], sq[:], ones128b[:],
                   start=(w == 0), stop=(w == NWIN - 1))

            # ---- phase 4: batchnorm ----
            statsb = sb.tile([64, 2], F32, tag="statsb")
            nc.scalar.copy(statsb[:], pStat[:])
            nc.sync.dma_start(out=cc_in[:, :], in_=statsb[:])
            nc.gpsimd.collective_compute(
                "AllReduce", OP.add, replica_groups=[list(range(NC))],
                ins=[cc_in[:, :]], outs=[cc_out[:, :]])
            statsg = sb.tile([64, 2], F32, tag="statsg")
            nc.sync.dma_start(out=statsg[:], in_=cc_out[:, :])
            fjs = sb.tile([64, 1], F32, tag="fjs")
            nc.vector.tensor_copy(fjs[:], statsg[:, 0:1])

            mean = sb.tile([64, 1], F32, tag="mean")
            nc.vector.tensor_scalar(mean[:], statsg[:, 0:1], 1.0 / N_DRUG,
                                    None, OP.mult)
            ex2 = sb.tile([64, 1], F32, tag="ex2")
            nc.vector.tensor_scalar(ex2[:], statsg[:, 1:2], 1.0 / N_DRUG,
                                    None, OP.mult)
            msq = sb.tile([64, 1], F32, tag="msq")
            nc.vector.tensor_tensor(out=msq[:], in0=mean[:], in1=mean[:],
                                    op=OP.mult)
            var = sb.tile([64, 1], F32, tag="var")
            nc.vector.tensor_tensor(out=var[:], in0=ex2[:], in1=msq[:],
                                    op=OP.subtract)
            vare = sb.tile([64, 1], F32, tag="vare")
            nc.vector.tensor_scalar(vare[:], var[:], EPS, None, OP.add)
            sd = sb.tile([64, 1], F32, tag="sd")
            nc.scalar.activation(sd[:], vare[:], AF.Sqrt)
            rstd = sb.tile([64, 1], F32, tag="rstd")
            nc.vector.reciprocal(rstd[:], sd[:])
            scalec = sb.tile([64, 1], F32, tag="scalec")
            nc.vector.tensor_tensor(out=scalec[:], in0=gcol[:], in1=rstd[:],
                                    op=OP.mult)
            tmp = sb.tile([64, 1], F32, tag="tmp")
            nc.vector.tensor_tensor(out=tmp[:], in0=mean[:], in1=scalec[:],
                                    op=OP.mult)
            shiftc = sb.tile([64, 1], F32, tag="shiftc")
            nc.vector.tensor_tensor(out=shiftc[:], in0=bcol[:], in1=tmp[:],
                                    op=OP.subtract)

            def col_to_tile(col, name, dtype=F32):
                pst = ps.tile([1, 64], F32, tag="ps")
                mm(pst[:], col[:], id64[:], is_transpose=True)
                row = sb.tile([1, 64], dtype, tag=name + "r")
                nc.scalar.copy(row[:], pst[:])
                t = cst.tile([128, 64], dtype, tag=name)
                nc.gpsimd.partition_broadcast(t[:], row[:])
                return t

            scale_t = col_to_tile(scalec, "scalet", BF16)
            shift_t = col_to_tile(shiftc, "shiftt", F32)
            for fi, ft in enumerate((scale_t, shift_t)):
                fjt = sb.tile([128, 1], F32, tag=f"fjt{fi}")
                nc.vector.tensor_copy(fjt[:], ft[:, 0:1])

            for w in range(NWIN):
                o1 = sb.tile([128, 64], F32, tag="o1")
                nc.vector.tensor_tensor(out=o1[:], in0=ybuf[:, w, :],
                                        in1=scale_t[:], op=OP.mult)
                o2 = sb.tile([128, 64], F32, tag="o2")
                nc.vector.tensor_tensor(out=o2[:], in0=o1[:], in1=shift_t[:],
                                        op=OP.add)
                nc.sync.dma_start(out=out_rows[w * 128:(w + 1) * 128, :],
                                  in_=o2[:])

        for _rep in range(int(os.environ.get('BASS_REPEAT', '1'))):
            body()

        for p in (psS, psU, psG, ps, wtp, selp, grp, swp, relp, ohtp,
                  tgp, tbp, sb, cst):
            p.release()

    nc.finalize()
    return nc


def _host_prep(DKG):
    """Sort edges by head, shard by head range, build per-core slot arrays
    (index-only preprocessing)."""
    heads = np.asarray(DKG[:, 0], dtype=np.int64)
    tails = np.asarray(DKG[:, 1], dtype=np.int64)
    rels = np.asarray(DKG[:, 2], dtype=np.int64)

    order = np.argsort(heads, kind="stable")
    hs, ts, rs = heads[order], tails[order], rels[order]

    core_lo = np.searchsorted(hs, HPC * np.arange(NC), side="left")
    core_hi = np.searchsorted(hs, HPC * (np.arange(NC) + 1), side="left")

    winb = np.searchsorted(hs, WIN * np.arange(NC * NWIN), side="left")
    wine = np.searchsorted(hs, WIN * (np.arange(NC * NWIN) + 1), side="left")
    maxw = int((wine - winb).max())
    NBW = max(1, (maxw + 127) // 128)
    NBW = (NBW + GEXT - 1) // GEXT * GEXT   # multiple of the extraction chunk
    NB = NWIN * NBW
    S = NB * 128

    oht_dt = np.dtype(mybir.dt.np(OHT_DT))
    rel_dt = np.dtype(mybir.dt.np(REL_DT))
    per_core = []
    for c in range(NC):
        lo, hi = core_lo[c], core_hi[c]
        ch, ct, cr = hs[lo:hi], ts[lo:hi], rs[lo:hi]
        hrel = np.full(S, PAD_H, np.float32)
        hrel_d = np.full(S, 127, np.float32)   # pads -> dummy head (finite sv)
        rel_d = np.zeros(S, np.int64)          # pads -> rel 0
        tail = np.zeros(S, np.int64)
        base = c * HPC
        for w in range(NWIN):
            wl = np.searchsorted(ch, base + w * WIN, side="left")
            wh = np.searchsorted(ch, base + (w + 1) * WIN, side="left")
            n = wh - wl
            o = w * NBW * 128
            hrel[o:o + n] = (ch[wl:wh] - base - w * WIN).astype(np.float32)
            hrel_d[o:o + n] = hrel[o:o + n]
            rel_d[o:o + n] = cr[wl:wh]
            tail[o:o + n] = ct[wl:wh]
        hrelC = hrel.reshape(NB, 128).T.copy()
        t16 = tail.reshape(S // 16, 16).T.astype(np.int16)          # [16, S/16]
        t16r = np.tile(t16, (8, 1)).copy()                          # [128, S/16]
        # ohT [head k, blk b, edge m] = (hrel_d[b*128+m] == k)
        hrel_bm = hrel_d.reshape(NB, 128)                           # [b, m]
        ohT = (np.arange(128, dtype=np.float32)[:, None, None]
               == hrel_bm[None, :, :])
        # relOH [edge p, blk b, r] = (rel_d[b*128+p] == r)
        rel_bp = rel_d.reshape(NB, 128).T                           # [p, b]
        relOH = (rel_bp[:, :, None]
                 == np.arange(64, dtype=np.int64)[None, None, :])
        # ohE [edge p, blk b, head k] = (hrel[b*128+p] == k); pads stay zero
        ohE = (hrelC[:, :, None]
               == np.arange(128, dtype=np.float32)[None, None, :])
        ohall = np.concatenate(
            [ohT.astype(oht_dt), ohE.astype(oht_dt), relOH.astype(oht_dt)],
            axis=2)
        per_core.append(dict(hrelC=hrelC, tails16=t16r, ohall=ohall))
    return NBW, per_core


def prepare(X, DKG, drug_emb, rel_emb, tail_emb, W1, b1, W2, b2, gamma, beta,
            W3, W4, W5):
    f = np.float32
    bf = np.dtype(mybir.dt.np(BF16))
    NBW, per_core = _host_prep(np.asarray(DKG))
    nc = _build_nc(NBW)

    consts = dict(
        c_iota128b=np.broadcast_to(np.arange(128, dtype=f),
                                   (128, 128)).astype(bf),
        c_id128=np.eye(128, dtype=f),
        c_id64=np.eye(64, dtype=f),
        c_id128b=np.eye(128, dtype=f).astype(bf),
        c_ones64=np.ones((64, 1), f),
        c_ones128b=np.ones((128, 1), f).astype(bf),
    )
    tp = np.zeros((TROWS, D), f)
    tp[:N_TAIL] = np.asarray(tail_emb, f)
    weights = dict(
        tail_pad=tp,
        rel_emb=np.asarray(rel_emb, f),
        W1=np.asarray(W1, f), W2=np.asarray(W2, f), W3=np.asarray(W3, f),
        W4=np.asarray(W4, f), W5=np.asarray(W5, f),
        b1c=np.asarray(b1, f).reshape(D, 1),
        b2r=np.asarray(b2, f).reshape(1, D),
        gammac=np.asarray(gamma, f).reshape(D, 1),
        betac=np.asarray(beta, f).reshape(D, 1),
    )
    de = np.asarray(drug_emb, f)
    in_maps = []
    for c in range(NC):
        dr = np.zeros((DROWS, D), f)
        dr[:HPC] = de[c * HPC:(c + 1) * HPC]
        m = dict(weights)
        m.update(consts)
        m["drug_rows"] = dr
        pc = per_core[c]
        m["hrelC"] = pc["hrelC"]
        m["tails16"] = pc["tails16"]
        m["ohall"] = pc["ohall"]
        in_maps.append(m)
    return nc, in_maps


def kernel(X, DKG, drug_emb, rel_emb, tail_emb, W1, b1, W2, b2, gamma, beta,
           W3, W4, W5):
    X = np.asarray(X)
    nc, in_maps = prepare(X, DKG, drug_emb, rel_emb, tail_emb, W1, b1, W2, b2,
                          gamma, beta, W3, W4, W5)

    res = run_bass_kernel_spmd(nc, in_maps, core_ids=list(range(NC)))
    global LAST_RESULT
    LAST_RESULT = res
    out = np.concatenate([np.asarray(res.results[c]["out_rows"][:HPC], np.float32)
                          for c in range(NC)], axis=0)
    return out, X


LAST_RESULT = None


# revision 18
# speedup vs baseline: 2.6811x; 1.1371x over previous
"""GAT layer (gnn_message_passing) Trainium2 kernel — v2.

Math (after algebraic simplification of the reference):
  v3 = W3 @ W5[:64];  v4 = W4 @ W5[64:]           # [64]
  s3 = drug_emb @ v3                               # [N_DRUG]
  s4 = tail_emb @ v4                               # [N_TAIL]
  Sb = drug_emb @ (rel_emb * (W1 @ 1)).T + sum(b1) # [N_DRUG, N_REL]
  att_e  = leaky_relu(s3[h_e] + s4[t_e])
  p_e    = exp(att_e)            (softmax max-shift dropped: shift-invariant)
  w_e    = p_e * Sb[h_e, r_e]
  U[h]   = sum_e w_e * tail_emb[t_e];  den[h] = sum_e p_e
  neigh  = U / den
  y      = [drug_emb | neigh] @ W2 + b2;  out = batchnorm(y) (training stats)

Sharding: edges sorted by head on the host (index-only preprocessing);
8 cores own disjoint 2500-head ranges, so segment stats complete locally.
Only the 64x2 batchnorm statistics are all-reduced.

v2 device strategy (per core, per 128-head window, 128-edge blocks):
  - tailB DRAM table [20480, 128] bf16 rows [tail|s4|pad], built on device
    once; per-window dma_gather pulls 256B rows (s4 rides along).
  - host ships one-hot matrices as fp8: ohT (head one-hot, lhsT for the
    per-edge SW-row gather matmul) and relOH (rel one-hot for Sb column
    selection).
  - per block: 1 pG matmul (gather [Sb_row|s3] per edge), 1 fused
    tensor_scalar builds the wg-scaled scatter one-hot, 1 pU matmul
    accumulates [U|den] in PSUM.  den uses rhs col64 = 1/sv so that
    wg*(1/sv) = p.
  - per 4 blocks: one TT-mult + tensor_reduce extracts sv; one TT-add
    forms att.  Per 16 blocks: lrelu (on DVE), exp (ACT), reciprocal etc.
  - all edge-pass matmuls bf16/fp8 (single HW pass vs fp32's two).
"""

import os

import numpy as np

import concourse.bacc as bacc
import concourse.bass as bass
import concourse.tile as tile
from concourse import mybir
from concourse.bass_utils import run_bass_kernel_spmd

F32 = mybir.dt.float32
BF16 = mybir.dt.bfloat16
F8 = mybir.dt.float8e4
I16 = mybir.dt.int16
AF = mybir.ActivationFunctionType
OP = mybir.AluOpType

N_DRUG = 20000
N_TAIL = 20000
N_REL = 64
D = 64
NC = 8
HPC = N_DRUG // NC          # heads per core
WIN = 128                   # heads per window
NWIN = (HPC + WIN - 1) // WIN  # windows per core (20)
DROWS = NWIN * WIN          # padded drug rows per core (2560)
TROWS = 20480               # padded tail rows (160 chunks of 128)
TCH = 16                    # tail-table chunks per iteration
EPS = 1e-5
SLOPE = 0.01
GEXT = 4                    # blocks per extraction chunk (PSUM-bank bound)
GRP = 16                    # blocks per batched-scalar group
NQ = int(os.environ.get("K_NQ", "1"))      # SWDGE queues for dma_gather
GCH = int(os.environ.get("K_GCH", "8"))    # blocks per dma_gather call
SIMIDX = os.environ.get("K_SIMIDX", "0") == "1"  # full-128 idx AP (CoreSim)
DMA_SCRATCH = int(os.environ.get("K_SCRATCH", "16384"))
PAD_H = 999.0               # hrel sentinel for padded slots (no one-hot match)
F8OHT = os.environ.get("K_F8OHT", "0") == "1"   # ship ohT as fp8 (else bf16)
F8REL = os.environ.get("K_F8REL", "0") == "1"   # ship relOH as fp8 (else bf16)
OHT_DT = F8 if F8OHT else BF16
REL_DT = F8 if F8REL else BF16


def _build_nc(NBW: int):
    """Build the Bass module. NBW = 128-edge blocks per 128-head window."""
    NB = NWIN * NBW          # blocks per core
    S = NB * 128             # edge slots per core

    nc = bacc.Bacc(None, num_devices=NC, num_swdge_queues=NQ,
                   dynamic_dma_scratch_size=DMA_SCRATCH)

    # ---- I/O ----
    def inp(name, shape, dtype=F32):
        return nc.declare_dram_parameter(name, list(shape), dtype, isOutput=False)

    tail_pad = inp("tail_pad", (TROWS, D))
    drug_rows = inp("drug_rows", (DROWS, D))
    rel_emb = inp("rel_emb", (N_REL, D))
    W1 = inp("W1", (D, D))
    W2 = inp("W2", (2 * D, D))
    W3 = inp("W3", (D, D))
    W4 = inp("W4", (D, D))
    W5 = inp("W5", (2 * D, 1))
    b1c = inp("b1c", (D, 1))
    b2r = inp("b2r", (1, D))
    gammac = inp("gammac", (D, 1))
    betac = inp("betac", (D, 1))

    hrelC = inp("hrelC", (128, NB))          # hrel per slot (pads = PAD_H)
    tails16 = inp("tails16", (128, S // 16), I16)
    ohT_in = inp("ohT", (128, NB, 128), OHT_DT)  # head one-hot [head, blk, edge]
    relOH_in = inp("relOH", (128, NB, 64), REL_DT)  # rel one-hot [edge, blk, rel]
    ohE_in = inp("ohE", (128, NB, 128), OHT_DT)  # scatter one-hot [edge, blk, head]

    c_iota128b = inp("c_iota128b", (128, 128), BF16)
    c_id128 = inp("c_id128", (128, 128))
    c_id64 = inp("c_id64", (64, 64))
    c_id128b = inp("c_id128b", (128, 128), BF16)
    c_ones64 = inp("c_ones64", (64, 1))
    c_ones128b = inp("c_ones128b", (128, 1), BF16)

    out_rows = nc.declare_dram_parameter("out_rows", [DROWS, D], F32, isOutput=True)

    # DRAM scratch: bf16 tail table rows [tail(64) | s4 | junk(63)]
    tailB = nc.dram_tensor("tailB", [TROWS, 128], BF16)

    # collective bounce buffers
    cc_in = nc.dram_tensor("cc_in", [D, 2], F32)
    cc_out = nc.dram_tensor("cc_out", [D, 2], F32, addr_space="Shared")

    with tile.TileContext(nc) as tc:
        cst = tc.alloc_tile_pool(name="cst", bufs=1)
        sb = tc.alloc_tile_pool(name="sb", bufs=2)
        tbp = tc.alloc_tile_pool(name="tbp", bufs=2)
        tgp = tc.alloc_tile_pool(name="tgp", bufs=2)
        ohtp = tc.alloc_tile_pool(name="ohtp", bufs=2)
        relp = tc.alloc_tile_pool(name="relp", bufs=2)
        swp = tc.alloc_tile_pool(name="swp", bufs=2)
        grp = tc.alloc_tile_pool(name="grp", bufs=2)
        selp = tc.alloc_tile_pool(name="selp", bufs=2)
        ohep = tc.alloc_tile_pool(name="ohep", bufs=2)
        wtp = tc.alloc_tile_pool(name="wtp", bufs=6)
        ps = tc.alloc_tile_pool(name="ps", bufs=1, space="PSUM")
        psG = tc.alloc_tile_pool(name="psG", bufs=3, space="PSUM")
        psU = tc.alloc_tile_pool(name="psU", bufs=2, space="PSUM")
        psS = tc.alloc_tile_pool(name="psS", bufs=1, space="PSUM")

        def body():
            # absorber: first DVE instruction after the init barrier must
            # carry no data wait.
            dve0 = cst.tile([128, 1], F32, tag="dve0")
            nc.vector.memset(dve0[:], 0.0)

            def mm(out, lhsT, rhs, start=True, stop=True, is_transpose=None):
                return nc.tensor.matmul(out, lhsT, rhs, start=start, stop=stop,
                                        is_transpose=is_transpose,
                                        skip_group_check=True)

            import bass_rust as _br

            def dep(a, b):
                _br.add_dep_helper(a.ins, b.ins, sync=True, reason="wait-routing")

            def load(pool, src_ap, shape, dtype=F32, name=None):
                t = pool.tile(list(shape), dtype, tag=name)
                nc.sync.dma_start(out=t[:], in_=src_ap)
                return t

            # ---- constants into SBUF ----
            iota128b = load(cst, c_iota128b[:, :], (128, 128), BF16, "iota128b")
            id128 = load(cst, c_id128[:, :], (128, 128), name="id128")
            id64 = load(cst, c_id64[:, :], (64, 64), name="id64")
            id128b = load(cst, c_id128b[:, :], (128, 128), BF16, "id128b")
            ones64 = load(cst, c_ones64[:, :], (64, 1), name="ones64")
            ones128b = load(cst, c_ones128b[:, :], (128, 1), BF16, "ones128b")

            hrelC_t = load(cst, hrelC[:, :], (128, NB), name="hrelC")
            tails_t = load(cst, tails16[:, :], (128, S // 16), I16, name="tails")

            w1t = load(cst, W1[:, :], (64, 64), name="w1")
            w2a = load(cst, W2[0:64, :], (64, 64), name="w2a")
            w2b = load(cst, W2[64:128, :], (64, 64), name="w2b")
            w3t = load(cst, W3[:, :], (64, 64), name="w3")
            w4t = load(cst, W4[:, :], (64, 64), name="w4")
            w5a = load(cst, W5[0:64, :], (64, 1), name="w5a")
            w5b = load(cst, W5[64:128, :], (64, 1), name="w5b")
            relt = load(cst, rel_emb[:, :], (64, 64), name="relt")
            b1col = load(cst, b1c[:, :], (64, 1), name="b1col")
            b2row = load(cst, b2r[:, :], (1, 64), name="b2row")
            gcol = load(cst, gammac[:, :], (64, 1), name="gcol")
            bcol = load(cst, betac[:, :], (64, 1), name="bcol")

            # ---- phase 1: weight folding (fp32, tiny) ----
            def transpose_to(pool, src_t, k, m, name):
                pst = ps.tile([m, k], F32, tag="ps")
                ident = id64 if k == 64 else id128
                mm(pst[:], src_t[:], ident[:, 0:k], is_transpose=True)
                dst = pool.tile([m, k], F32, tag=name)
                nc.scalar.copy(dst[:], pst[:])
                return dst

            w3T = transpose_to(cst, w3t, 64, 64, "w3T")
            w4T = transpose_to(cst, w4t, 64, 64, "w4T")
            w1T = transpose_to(cst, w1t, 64, 64, "w1T")
            relT = transpose_to(cst, relt, 64, 64, "relT")

            def mm_to_sbuf(pool, lhsT, rhs, m, n, name, dtype=F32):
                pst = ps.tile([m, n], F32, tag="ps")
                mm(pst[:], lhsT, rhs)
                dst = pool.tile([m, n], dtype, tag=name)
                nc.scalar.copy(dst[:], pst[:])
                return dst

            v3b = mm_to_sbuf(cst, w3T[:], w5a[:], 64, 1, "v3b", BF16)   # [64,1]
            v4 = mm_to_sbuf(cst, w4T[:], w5b[:], 64, 1, "v4")           # [64,1]
            w1s = mm_to_sbuf(cst, w1T[:], ones64[:], 64, 1, "w1s")      # [64,1]
            b1s = mm_to_sbuf(cst, b1col[:], ones64[:], 1, 1, "b1s")     # [1,1]
            # +1e-12 biases Sb so the per-edge reciprocal below never hits 0
            b1sp = cst.tile([1, 1], F32, tag="b1sp")
            nc.vector.tensor_scalar(b1sp[:], b1s[:], 1e-12, None, OP.add)

            # v4 broadcast tile [128, 64] fp32 (for the tail-table build)
            psv4r = ps.tile([1, 64], F32, tag="ps")
            mm(psv4r[:], v4[:], id64[:], is_transpose=True)
            v4row = cst.tile([1, 64], F32, tag="v4row")
            nc.scalar.copy(v4row[:], psv4r[:])
            v4tile = cst.tile([128, 64], F32, tag="v4tile")
            nc.gpsimd.partition_broadcast(v4tile[:], v4row[:])

            b1s_tile = cst.tile([128, 1], F32, tag="b1stile")
            nc.gpsimd.partition_broadcast(b1s_tile[:], b1sp[:])

            b2tile = cst.tile([128, 64], F32, tag="b2tile")
            nc.gpsimd.partition_broadcast(b2tile[:], b2row[:])

            # bf16 weights for the edge/output matmuls
            w2ab = cst.tile([64, 64], BF16, tag="w2ab")
            nc.scalar.copy(w2ab[:], w2a[:])
            w2bb = cst.tile([64, 64], BF16, tag="w2bb")
            nc.scalar.copy(w2bb[:], w2b[:])

            # M_T = rel_emb.T * w1s (per-partition scale), bf16
            MTb = cst.tile([64, 64], BF16, tag="MTb")
            nc.vector.tensor_scalar(MTb[:], relT[:], w1s[:], None, OP.mult)

            # DVE fences: absorb preload DMA waits so hot-loop compact DVE
            # ops never need more than one embedded wait.
            for fi, ft in enumerate((iota128b, hrelC_t, v4tile, b2tile,
                                     gcol, bcol)):
                np_ = ft.shape[0]
                fj = cst.tile([np_, 1], F32, tag=f"fj{fi}")
                nc.vector.tensor_copy(fj[:], ft[0:np_, 0:1])
            fj16 = cst.tile([128, 1], I16, tag="fj16")
            nc.vector.tensor_copy(fj16[:], tails_t[:, 0:1])

            # ---- phase 1b: tail table build ----
            NIT = TROWS // (TCH * 128)
            for it in range(NIT):
                base = it * TCH * 128
                tch = tbp.tile([128, TCH, 64], F32, tag="tch")
                src = tail_pad[base:base + TCH * 128, :].rearrange(
                    "(c p) d -> p c d", p=128)
                nc.sync.dma_start(out=tch[:], in_=src)
                tbb = tbp.tile([128, TCH, 128], BF16, tag="tbb")
                nc.vector.memset(tbb[:, :, 65:128], 0.0)
                s4f = tbp.tile([128, TCH], F32, tag="s4f")
                for c in range(TCH):
                    junk = sb.tile([128, 64], F32, tag="junk")
                    nc.vector.scalar_tensor_tensor(
                        out=junk[:], in0=tch[:, c, :], scalar=0.0,
                        in1=v4tile[:], op0=OP.bypass, op1=OP.mult,
                        accum_out=s4f[:, c:c + 1])
                nc.vector.tensor_copy(tbb[:, :, 0:64], tch[:])
                nc.vector.tensor_copy(tbb[:, :, 64], s4f[:])
                dst = tailB[base:base + TCH * 128, :].rearrange(
                    "(c p) d -> p c d", p=128)
                nc.sync.dma_start(out=dst, in_=tbb[:])

            # ---- phase 2+3 fused: per-window pipeline ----
            ybuf = cst.tile([128, NWIN, 64], BF16, tag="ybuf")
            pStat = psS.tile([64, 2], F32, tag="pStat")
            drugTs = []

            NCH = (NBW + GCH - 1) // GCH
            for w in range(NWIN):
                # window drug prep -> SW = [Sb | s3] bf16 [128, 65]
                dchunk = sb.tile([128, 64], F32, tag="dchunk")
                nc.sync.dma_start(out=dchunk[:],
                                  in_=drug_rows[w * 128:(w + 1) * 128, :])
                psDT = ps.tile([64, 128], F32, tag="ps")
                mm(psDT[:], dchunk[:], id128[:], is_transpose=True)
                dTb = cst.tile([64, 128], BF16, tag=f"drugT{w}")
                nc.scalar.copy(dTb[:], psDT[:])
                drugTs.append(dTb)
                SW = swp.tile([128, 65], BF16, tag="SW")
                psSb = ps.tile([128, 64], F32, tag="ps")
                mm(psSb[:], dTb[:], MTb[:])
                nc.scalar.activation(SW[:, 0:64], psSb[:], AF.Identity,
                                     bias=b1s_tile[:], scale=1.0)
                psS3 = ps.tile([128, 1], F32, tag="ps")
                mm(psS3[:], dTb[:], v3b[:])
                nc.scalar.copy(SW[:, 64:65], psS3[:])

                # gather this window's tail rows [128, NBW, 128] bf16
                tg = tgp.tile([128, NBW, 128], BF16, tag="tg")
                gat_i = None
                for g0 in range(0, NBW, GCH):
                    gn = min(GCH, NBW - g0) * 128
                    io = (w * NBW + g0) * 8
                    gat_i = nc.gpsimd.dma_gather(
                        out_ap=tg[:, g0:g0 + gn // 128, :],
                        in_ap=tailB[:, :],
                        idxs_ap=(tails_t[:, io:io + gn // 16] if SIMIDX
                                 else tails_t[0:16, io:io + gn // 16]),
                        num_idxs=gn,
                        num_idxs_reg=gn,
                        elem_size=128,
                        queue_num=(w * NCH + g0 // GCH) % NQ,
                    )

                # stream this window's one-hots
                ohTw = ohtp.tile([128, NBW, 128], OHT_DT, tag="ohTw")
                nc.sync.dma_start(out=ohTw[:],
                                  in_=ohT_in[:, w * NBW:(w + 1) * NBW, :])
                relw = relp.tile([128, NBW, 64], REL_DT, tag="relw")
                nc.sync.dma_start(out=relw[:],
                                  in_=relOH_in[:, w * NBW:(w + 1) * NBW, :])
                ohEw = ohep.tile([128, NBW, 128], OHT_DT, tag="ohEw")
                nc.sync.dma_start(out=ohEw[:],
                                  in_=ohE_in[:, w * NBW:(w + 1) * NBW, :])

                pU = psU.tile([128, 65], F32, tag="pU")

                for j0 in range(0, NBW, GRP):
                    g = min(GRP, NBW - j0)
                    sv16 = grp.tile([128, GRP], F32, tag="sv16")
                    att16 = grp.tile([128, GRP], F32, tag="att16")
                    s4c16 = grp.tile([128, GRP], F32, tag="s4c16")
                    # s4 junction copy (absorbs the gather DMA wait)
                    s4i = nc.scalar.copy(s4c16[:, 0:g], tg[:, j0:j0 + g, 64])
                    dep(s4i, gat_i)
                    for q in range(0, g, GEXT):
                        psG4 = psG.tile([128, GEXT, 65], F32, tag="psG4")
                        for i in range(GEXT):
                            j = j0 + q + i
                            mm(psG4[:, i, :], ohTw[:, j, :], SW[:],
                               start=True, stop=True)
                        sel4 = selp.tile([128, GEXT, 64], BF16, tag="sel4")
                        nc.vector.tensor_tensor(
                            out=sel4[:], in0=psG4[:, :, 0:64],
                            in1=relw[:, j0 + q:j0 + q + GEXT, :], op=OP.mult)
                        nc.vector.tensor_reduce(
                            out=sv16[:, q:q + GEXT], in_=sel4[:],
                            axis=mybir.AxisListType.X, op=OP.add)
                        nc.vector.tensor_tensor(
                            out=att16[:, q:q + GEXT], in0=psG4[:, :, 64],
                            in1=s4c16[:, q:q + GEXT], op=OP.add)
                    # group scalar pipeline [128, g]
                    l16 = grp.tile([128, GRP], F32, tag="l16")
                    nc.vector.scalar_tensor_tensor(
                        out=l16[:, 0:g], in0=att16[:, 0:g], scalar=SLOPE,
                        in1=att16[:, 0:g], op0=OP.mult, op1=OP.max)
                    p16 = grp.tile([128, GRP], F32, tag="p16")
                    nc.scalar.activation(p16[:, 0:g], l16[:, 0:g], AF.Exp)
                    wg16 = grp.tile([128, GRP], F32, tag="wg16")
                    nc.vector.tensor_tensor(out=wg16[:, 0:g], in0=p16[:, 0:g],
                                            in1=sv16[:, 0:g], op=OP.mult)
                    rc16 = grp.tile([128, GRP], F32, tag="rc16")
                    nc.vector.reciprocal(rc16[:, 0:g], sv16[:, 0:g])
                    nc.scalar.copy(tg[:, j0:j0 + g, 64], rc16[:, 0:g])
                    for jj in range(g):
                        j = j0 + jj
                        wt = wtp.tile([128, 65], BF16, tag="wt")
                        nc.scalar.activation(wt[:], tg[:, j, 0:65], AF.Identity,
                                             scale=wg16[:, jj:jj + 1])
                        mm(pU[:], ohEw[:, j, :], wt[:],
                           start=(j == 0), stop=(j == NBW - 1))

                # window reduction -> neigh -> y -> stats
                dsafe = sb.tile([128, 1], F32, tag="dsafe")
                nc.vector.tensor_scalar(dsafe[:], pU[:, 64:65], 1e-30, None,
                                        OP.add)
                recip = sb.tile([128, 1], F32, tag="recip")
                nc.vector.reciprocal(recip[:], dsafe[:])
                nw = sb.tile([128, 64], F32, tag="nw")
                nc.vector.tensor_scalar(nw[:], pU[:, 0:64], recip[:], None,
                                        OP.mult)
                psNT = ps.tile([64, 128], F32, tag="psT")
                mm(psNT[:], nw[:], id128[:], is_transpose=True)
                nT = sb.tile([64, 128], BF16, tag="nT")
                nc.scalar.copy(nT[:], psNT[:])
                pY = ps.tile([128, 64], F32, tag="ps")
                mm(pY[:], drugTs[w][:], w2ab[:], start=True, stop=False)
                mm(pY[:], nT[:], w2bb[:], start=False, stop=True)
                nc.vector.tensor_tensor(out=ybuf[:, w, :], in0=pY[:],
                                        in1=b2tile[:], op=OP.add)
                sq = sb.tile([128, 64], BF16, tag="sq")
                nc.vector.tensor_tensor(out=sq[:], in0=ybuf[:, w, :],
                                        in1=ybuf[:, w, :], op=OP.mult)
                mm(pStat[:, 0:1], ybuf[:, w, :], ones128b[:],
                   start=(w == 0), stop=(w == NWIN - 1))
                mm(pStat[:, 1:2], sq[:], ones128b[:],
                   start=(w == 0), stop=(w == NWIN - 1))

            # ---- phase 4: batchnorm ----
            statsb = sb.tile([64, 2], F32, tag="statsb")
            nc.scalar.copy(statsb[:], pStat[:])
            nc.sync.dma_start(out=cc_in[:, :], in_=statsb[:])
            nc.gpsimd.collective_compute(
                "AllReduce", OP.add, replica_groups=[list(range(NC))],
                ins=[cc_in[:, :]], outs=[cc_out[:, :]])
            statsg = sb.tile([64, 2], F32, tag="statsg")
            nc.sync.dma_start(out=statsg[:], in_=cc_out[:, :])
            fjs = sb.tile([64, 1], F32, tag="fjs")
            nc.vector.tensor_copy(fjs[:], statsg[:, 0:1])

            mean = sb.tile([64, 1], F32, tag="mean")
            nc.vector.tensor_scalar(mean[:], statsg[:, 0:1], 1.0 / N_DRUG,
                                    None, OP.mult)
            ex2 = sb.tile([64, 1], F32, tag="ex2")
            nc.vector.tensor_scalar(ex2[:], statsg[:, 1:2], 1.0 / N_DRUG,
                                    None, OP.mult)
            msq = sb.tile([64, 1], F32, tag="msq")
            nc.vector.tensor_tensor(out=msq[:], in0=mean[:], in1=mean[:],
                                    op=OP.mult)
            var = sb.tile([64, 1], F32, tag="var")
            nc.vector.tensor_tensor(out=var[:], in0=ex2[:], in1=msq[:],
                                    op=OP.subtract)
            vare = sb.tile([64, 1], F32, tag="vare")
            nc.vector.tensor_scalar(vare[:], var[:], EPS, None, OP.add)
            sd = sb.tile([64, 1], F32, tag="sd")
            nc.scalar.activation(sd[:], vare[:], AF.Sqrt)
            rstd = sb.tile([64, 1], F32, tag="rstd")
            nc.vector.reciprocal(rstd[:], sd[:])
            scalec = sb.tile([64, 1], F32, tag="scalec")
            nc.vector.tensor_tensor(out=scalec[:], in0=gcol[:], in1=rstd[:],
                                    op=OP.mult)
            tmp = sb.tile([64, 1], F32, tag="tmp")
            nc.vector.tensor_tensor(out=tmp[:], in0=mean[:], in1=scalec[:],
                                    op=OP.mult)
            shiftc = sb.tile([64, 1], F32, tag="shiftc")
            nc.vector.tensor_tensor(out=shiftc[:], in0=bcol[:], in1=tmp[:],
                                    op=OP.subtract)

            def col_to_tile(col, name, dtype=F32):
                pst = ps.tile([1, 64], F32, tag="ps")
                mm(pst[:], col[:], id64[:], is_transpose=True)
                row = sb.tile([1, 64], dtype, tag=name + "r")
                nc.scalar.copy(row[:], pst[:])
                t = cst.tile([128, 64], dtype, tag=name)
                nc.gpsimd.partition_broadcast(t[:], row[:])
                return t

            scale_t = col_to_tile(scalec, "scalet", BF16)
            shift_t = col_to_tile(shiftc, "shiftt", F32)
            for fi, ft in enumerate((scale_t, shift_t)):
                fjt = sb.tile([128, 1], F32, tag=f"fjt{fi}")
                nc.vector.tensor_copy(fjt[:], ft[:, 0:1])

            for w in range(NWIN):
                o1 = sb.tile([128, 64], F32, tag="o1")
                nc.vector.tensor_tensor(out=o1[:], in0=ybuf[:, w, :],
                                        in1=scale_t[:], op=OP.mult)
                o2 = sb.tile([128, 64], F32, tag="o2")
                nc.vector.tensor_tensor(out=o2[:], in0=o1[:], in1=shift_t[:],
                                        op=OP.add)
                nc.sync.dma_start(out=out_rows[w * 128:(w + 1) * 128, :],
                                  in_=o2[:])

        for _rep in range(int(os.environ.get('BASS_REPEAT', '1'))):
            body()

        for p in (psS, psU, psG, ps, wtp, ohep, selp, grp, swp, relp, ohtp,
                  tgp, tbp, sb, cst):
            p.release()

    nc.finalize()
    return nc


def _host_prep(DKG):
    """Sort edges by head, shard by head range, build per-core slot arrays
    (index-only preprocessing)."""
    heads = np.asarray(DKG[:, 0], dtype=np.int64)
    tails = np.asarray(DKG[:, 1], dtype=np.int64)
    rels = np.asarray(DKG[:, 2], dtype=np.int64)

    order = np.argsort(heads, kind="stable")
    hs, ts, rs = heads[order], tails[order], rels[order]

    core_lo = np.searchsorted(hs, HPC * np.arange(NC), side="left")
    core_hi = np.searchsorted(hs, HPC * (np.arange(NC) + 1), side="left")

    winb = np.searchsorted(hs, WIN * np.arange(NC * NWIN), side="left")
    wine = np.searchsorted(hs, WIN * (np.arange(NC * NWIN) + 1), side="left")
    maxw = int((wine - winb).max())
    NBW = max(1, (maxw + 127) // 128)
    NBW = (NBW + GEXT - 1) // GEXT * GEXT   # multiple of the extraction chunk
    NB = NWIN * NBW
    S = NB * 128

    oht_dt = np.dtype(mybir.dt.np(OHT_DT))
    rel_dt = np.dtype(mybir.dt.np(REL_DT))
    per_core = []
    for c in range(NC):
        lo, hi = core_lo[c], core_hi[c]
        ch, ct, cr = hs[lo:hi], ts[lo:hi], rs[lo:hi]
        hrel = np.full(S, PAD_H, np.float32)
        hrel_d = np.full(S, 127, np.float32)   # pads -> dummy head (finite sv)
        rel_d = np.zeros(S, np.int64)          # pads -> rel 0
        tail = np.zeros(S, np.int64)
        base = c * HPC
        for w in range(NWIN):
            wl = np.searchsorted(ch, base + w * WIN, side="left")
            wh = np.searchsorted(ch, base + (w + 1) * WIN, side="left")
            n = wh - wl
            o = w * NBW * 128
            hrel[o:o + n] = (ch[wl:wh] - base - w * WIN).astype(np.float32)
            hrel_d[o:o + n] = hrel[o:o + n]
            rel_d[o:o + n] = cr[wl:wh]
            tail[o:o + n] = ct[wl:wh]
        hrelC = hrel.reshape(NB, 128).T.copy()
        t16 = tail.reshape(S // 16, 16).T.astype(np.int16)          # [16, S/16]
        t16r = np.tile(t16, (8, 1)).copy()                          # [128, S/16]
        # ohT [head k, blk b, edge m] = (hrel_d[b*128+m] == k)
        hrel_bm = hrel_d.reshape(NB, 128)                           # [b, m]
        ohT = (np.arange(128, dtype=np.float32)[:, None, None]
               == hrel_bm[None, :, :])
        # relOH [edge p, blk b, r] = (rel_d[b*128+p] == r)
        rel_bp = rel_d.reshape(NB, 128).T                           # [p, b]
        relOH = (rel_bp[:, :, None]
                 == np.arange(64, dtype=np.int64)[None, None, :])
        # ohE [edge p, blk b, head k] = (hrel[b*128+p] == k); pads stay zero
        ohE = (hrelC[:, :, None]
               == np.arange(128, dtype=np.float32)[None, None, :])
        per_core.append(dict(hrelC=hrelC, tails16=t16r,
                             ohT=ohT.astype(oht_dt),
                             relOH=relOH.astype(rel_dt),
                             ohE=ohE.astype(oht_dt)))
    return NBW, per_core


def prepare(X, DKG, drug_emb, rel_emb, tail_emb, W1, b1, W2, b2, gamma, beta,
            W3, W4, W5):
    f = np.float32
    bf = np.dtype(mybir.dt.np(BF16))
    NBW, per_core = _host_prep(np.asarray(DKG))
    nc = _build_nc(NBW)

    consts = dict(
        c_iota128b=np.broadcast_to(np.arange(128, dtype=f),
                                   (128, 128)).astype(bf),
        c_id128=np.eye(128, dtype=f),
        c_id64=np.eye(64, dtype=f),
        c_id128b=np.eye(128, dtype=f).astype(bf),
        c_ones64=np.ones((64, 1), f),
        c_ones128b=np.ones((128, 1), f).astype(bf),
    )
    tp = np.zeros((TROWS, D), f)
    tp[:N_TAIL] = np.asarray(tail_emb, f)
    weights = dict(
        tail_pad=tp,
        rel_emb=np.asarray(rel_emb, f),
        W1=np.asarray(W1, f), W2=np.asarray(W2, f), W3=np.asarray(W3, f),
        W4=np.asarray(W4, f), W5=np.asarray(W5, f),
        b1c=np.asarray(b1, f).reshape(D, 1),
        b2r=np.asarray(b2, f).reshape(1, D),
        gammac=np.asarray(gamma, f).reshape(D, 1),
        betac=np.asarray(beta, f).reshape(D, 1),
    )
    de = np.asarray(drug_emb, f)
    in_maps = []
    for c in range(NC):
        dr = np.zeros((DROWS, D), f)
        dr[:HPC] = de[c * HPC:(c + 1) * HPC]
        m = dict(weights)
        m.update(consts)
        m["drug_rows"] = dr
        pc = per_core[c]
        m["hrelC"] = pc["hrelC"]
        m["tails16"] = pc["tails16"]
        m["ohT"] = pc["ohT"]
        m["relOH"] = pc["relOH"]
        m["ohE"] = pc["ohE"]
        in_maps.append(m)
    return nc, in_maps


def kernel(X, DKG, drug_emb, rel_emb, tail_emb, W1, b1, W2, b2, gamma, beta,
           W3, W4, W5):
    X = np.asarray(X)
    nc, in_maps = prepare(X, DKG, drug_emb, rel_emb, tail_emb, W1, b1, W2, b2,
                          gamma, beta, W3, W4, W5)

    res = run_bass_kernel_spmd(nc, in_maps, core_ids=list(range(NC)))
    global LAST_RESULT
    LAST_RESULT = res
    out = np.concatenate([np.asarray(res.results[c]["out_rows"][:HPC], np.float32)
                          for c in range(NC)], axis=0)
    return out, X


LAST_RESULT = None
